# revision 19
# baseline (speedup 1.0000x reference)
"""Transformer encoder layer (nn_Encoder) on 8 TRN2 NeuronCores.

Strategy: data-parallel over batch — B=8, one batch element per core, weights
replicated, no collectives. Per core a single Bass/Tile kernel computes the
whole layer.

Precision plan: the attention path (QKV projections, softmax-weighted context,
Wo) runs in fp8e4m3 with DoubleRow matmuls (2 contraction tiles per
instruction, 0.5 cycles/row) — its error is attenuated ~25x by the residual
(|attn_out| ~ 0.04 |x|). Attention scores stay bf16. The FFN (whose output is
NOT small vs the residual) runs in bf16 at 1 cycle/row. All accumulation in
fp32 PSUM. fp8 operands are host-prescaled by 32 (weights are sigma~1/32 and
would hit the fp8 denormal floor); the combined 1024x scale on the attention
output is folded into the residual (x side is scaled 1024x instead) and
neutralized by LayerNorm's scale invariance.

Layout: attention runs in the "transposed domain" ([feature, tokens]) so every
weight matmul uses natural weight layouts; softmax over tokens-on-partitions is
handled by appending a ones-column to V (denominator lands in the ctx matmul's
extra output row, M=65). Per pair the 4 denominator rows are staged into one
[4, NS] tile, inverted with a single reciprocal, broadcast across partitions
with a tiny K=4 matmul against a selection matrix, and applied in one
full-width multiply per slice (deferred one pair to keep PE fed). Wo/FFN2
products land in the natural domain where both LayerNorms reduce along the
free dim; the normalize runs on ACT via per-partition scale/bias. b2 is folded
into be1 on the host (with b1 -= W1^T b2 compensating FFN1). W1/Wo are
prefetched into SBUF during the attention phase and W2 during FFN1, each read
from HBM exactly once. FFN2 runs si-major with per-si epilogues so only the
last tile's epilogue trails the final matmul.

Self-contained: hardcodes B=8, S=1024, D=1024, H=16, FF=2048, 8 cores.
"""
import math
import numpy as np
import ml_dtypes
from contextlib import ExitStack

import concourse.bass as bass
import concourse.tile as tile
from concourse import bacc, mybir
from concourse import bass_utils
from concourse.masks import make_identity

B = 8
S = 1024
D = 1024
H = 16
FF = 2048
P = 128
HD = 64
EPS = 1e-5
f32 = mybir.dt.float32
f32r = mybir.dt.float32r
bf16 = mybir.dt.bfloat16
f8 = mybir.dt.float8e4
np_bf16 = ml_dtypes.bfloat16
np_f8 = ml_dtypes.float8_e4m3
DR = mybir.MatmulPerfMode.DoubleRow
AF = mybir.ActivationFunctionType
ALU = mybir.AluOpType

NP_ = H // 2          # head pairs
ST = S // P           # token tiles
DT = D // P
D2 = DT // 2          # contraction pair-tiles over D
T2 = ST // 2          # token pair-tiles
FT = FF // P
NS = 512              # token slice width (matmul free dim)
SL = S // NS
ND = 512              # feature slice width
DL = D // ND

WSC = 32.0            # fp8 weight prescale (weights are sigma ~ 1/32)
RSC = WSC * WSC       # resulting scale on the attention output


def build_encoder(num_devices=8):
    # q,k arrive prescaled by WSC each: scores carry WSC^2; exp folds it into
    # its input scale. The extra -2ln2 bias keeps exp outputs < 240/4 (fp8e4m3
    # max is 240); softmax cancels any constant factor on e.
    e_scale = 1.0 / (math.sqrt(HD) * RSC)
    e_bias = -2.0 * math.log(2.0)
    nc = bacc.Bacc("TRN2", target_bir_lowering=False, debug=False,
                   enable_asserts=True, num_devices=num_devices)

    dram = lambda n, sh, dt: nc.dram_tensor(n, sh, dt, kind="ExternalInput").ap()
    xT_d = dram("xT", [D2, P, 2, S], f8)
    vones_d = dram("vones", [P, H], f8)
    sel_d = dram("sel", [SL, 4, P], f32r)
    x_d = dram("x", [S, D], f32)
    wq_d = dram("Wq", [NP_, P, D2, 2, P], f8)
    wk_d = dram("Wk", [NP_, P, D2, 2, P], f8)
    wv_d = dram("Wv", [D2, P, 2, D], f8)
    wo_d = dram("Wo", [D2, P, 2, D], f8)
    w1_d = dram("W1", [FT, P, DT, P], bf16)
    w2_d = dram("W2", [FF, D], bf16)
    bqc_d = dram("bqc", [P, NP_], f32)
    bkc_d = dram("bkc", [P, NP_], f32)
    b1c_d = dram("b1c", [P, FT], f32)
    bv_d = dram("bv", [D], f32)
    bo_d = dram("bo", [D], f32)
    g1_d = dram("g1", [D], f32)
    be1_d = dram("be1", [D], f32)
    g2_d = dram("g2", [D], f32)
    be2_d = dram("be2", [D], f32)
    out_d = nc.dram_tensor("out", [S, D], f32, kind="ExternalOutput").ap()

    with tile.TileContext(nc) as tc, ExitStack() as octx:
        const = octx.enter_context(tc.tile_pool(name="const", bufs=1))
        identity = const.tile([P, P], bf16, name="identity")
        make_identity(nc, identity)
        ebias = const.tile([P, 1], f32, name="ebias")
        nc.gpsimd.memset(ebias[:], e_bias)

        # long-lived weight pool (prefetched during attention) + h tiles
        pW = octx.enter_context(tc.tile_pool(name="pW", bufs=1))
        pH = octx.enter_context(tc.tile_pool(name="pH", bufs=1))

        def bcast_row(pool, name, src_row, width):
            r = pool.tile([1, width], f32, name=f"{name}_r", tag="bcr", bufs=1)
            nc.sync.dma_start(r[:], src_row[None, :])
            b = pool.tile([P, width], f32, name=f"{name}_b", tag=f"{name}_b")
            nc.gpsimd.partition_broadcast(b[:], r[:])
            return b

        jview = lambda ap, w: ap.rearrange("p (j c) -> p j c", j=2)

        # ctxT pool (attention -> Wo), fp8 dv-pair tiles [P, 2, S]
        pCtx_cm = tc.tile_pool(name="pCtx", bufs=1)
        pCtx = pCtx_cm.__enter__()

        # ---------------- attention scope ----------------
        with tc.tile_pool(name="pA", bufs=1) as pA, \
             tc.tile_pool(name="psA", bufs=1, space="PSUM") as psA:

            # pair-0 Q/K weights first so QK(0) matmuls start ASAP
            wq0 = pA.tile([P, D2 * 2 * P], f8, name="wq0", tag="wq", bufs=2)
            nc.sync.dma_start(
                wq0[:].rearrange("p (d j m) -> p d j m", j=2, m=P), wq_d[0])
            wk0 = pA.tile([P, D2 * 2 * P], f8, name="wk0", tag="wk", bufs=2)
            nc.sync.dma_start(
                wk0[:].rearrange("p (d j m) -> p d j m", j=2, m=P), wk_d[0])

            # x^T fp8 pair tiles [P, 2, S]
            xt2 = []
            for d2 in range(D2):
                t = pA.tile([P, 2 * S], f8, name=f"xt{d2}", tag="xt", bufs=D2)
                nc.sync.dma_start(jview(t, S), xT_d[d2])
                xt2.append(t)

            # ---- V projection weights ----
            pExp_cm = tc.tile_pool(name="pExp", bufs=1)
            pExp = pExp_cm.__enter__()
            pV_cm = tc.tile_pool(name="pV", bufs=1)
            pV = pV_cm.__enter__()
            wv2 = []
            for d2 in range(D2):
                t = pV.tile([P, 2 * D], f8, name=f"wv{d2}", tag="wv", bufs=D2)
                nc.sync.dma_start(jview(t, D), wv_d[d2])
                wv2.append(t)

            # V65 token-pair tiles: [128, 2, H*65], ones at [:, :, 65h+64]
            v65 = []
            for t2 in range(T2):
                v = pA.tile([P, 2 * H * 65], f8, name=f"v65_{t2}", tag="v65",
                            bufs=T2)
                for j in range(2):
                    nc.sync.dma_start(
                        jview(v, H * 65).rearrange(
                            "p j (h c) -> p j h c", c=65)[:, j, :, 64:65],
                        vones_d[:, :, None])
                v65.append(v)

            # small consts (needed only after the first QK chains land)
            bqc = const.tile([P, NP_], f32, name="bqc")
            nc.sync.dma_start(bqc[:], bqc_d)
            bkc = const.tile([P, NP_], f32, name="bkc")
            nc.sync.dma_start(bkc[:], bkc_d)
            b1c = const.tile([P, FT], f32, name="b1c")
            nc.sync.dma_start(b1c[:], b1c_d)
            selt = const.tile([4, SL * P], f32r, name="selt")
            for sl in range(SL):
                nc.sync.dma_start(selt[:, sl * P:(sl + 1) * P], sel_d[sl])
            bv_b = bcast_row(pA, "bv", bv_d, D)

            # Wo prefetch (fp8 dv-pair tiles [P, 2, D])
            wo2 = []
            for q2 in range(D2):
                t = pW.tile([P, 2 * D], f8, name=f"wo{q2}", tag="wo", bufs=D2)
                nc.sync.dma_start(jview(t, D), wo_d[q2])
                wo2.append(t)

            # W1 resident tiles; DMAs emitted later in the pair loop
            w1 = [pW.tile([P, DT * P], bf16, name=f"w1_{f}", tag="w1", bufs=FT)
                  for f in range(FT)]

            def emit_w1_loads():
                for f in range(FT):
                    nc.sync.dma_start(
                        w1[f][:].rearrange("p (dt q) -> p dt q", q=P),
                        w1_d[f])

            hpn = ND // HD
            v_state = {}

            def emit_v_chunk(hc):
                """Half-chunk hc of the V projection (chain = hc//2)."""
                chain = hc // 2
                part = hc % 2
                t, n = chain // DL, chain % DL
                if part == 0:
                    v_state[chain] = psA.tile(
                        [P, ND], f32, name=f"vps{t}_{n}", tag="vqk", bufs=2)
                ps = v_state[chain]
                for d2 in range(2 * part, 2 * part + 2):
                    nc.tensor.matmul(
                        ps[:], jview(xt2[d2], S)[:, :, t * P:(t + 1) * P],
                        jview(wv2[d2], D)[:, :, n * ND:(n + 1) * ND],
                        perf_mode=DR, start=(d2 == 0), stop=(d2 == D2 - 1))
                if part == 1:
                    dst = jview(v65[t // 2], H * 65).rearrange(
                        "p j (h c) -> p j h c", c=65)[
                        :, t % 2, n * hpn:(n + 1) * hpn, 0:64]
                    srcv = ps[:].rearrange("p (h k) -> p h k", k=HD)
                    bvs = bv_b[:, n * ND:(n + 1) * ND].rearrange(
                        "p (h k) -> p h k", k=HD)
                    nc.vector.tensor_add(dst, srcv, bvs)

            # ---- attention per head pair ----
            ctxT2 = [pCtx.tile([P, 2 * S], f8, name=f"ctxT{q2}", tag="ctxT",
                               bufs=D2) for q2 in range(D2)]

            def emit_normalize(p, ctxU, den4):
                """Deferred softmax-normalize of pair p's ctx."""
                den4r = pA.tile([4, NS], f32r, name=f"den4r_{p}", tag="den4r",
                                bufs=2)
                with nc.allow_low_precision("softmax denom recip in f32r"):
                    nc.vector.reciprocal(den4r[:], den4[:])
                for sl in range(SL):
                    rcb = psA.tile([P, NS], f32, name=f"rcb{p}_{sl}",
                                   tag="vqk", bufs=2)
                    nc.tensor.matmul(rcb[:], selt[:, sl * P:(sl + 1) * P],
                                     den4r[:], start=True, stop=True)
                    nc.vector.tensor_mul(
                        jview(ctxT2[p // 2], S)[
                            :, p % 2, sl * NS:(sl + 1) * NS],
                        ctxU[:, sl * NS:(sl + 1) * NS], rcb[:])

            def emit_qk_chain_part(p, chain, part, state):
                """Emit 2 of the 4 DoubleRow accumulation matmuls of QK chain
                (chain: 0..3 = Q-sl0, Q-sl1, K-sl0, K-sl1) for pair p."""
                wt, bc, dst = state["ops"][chain // 2]
                sl = chain % 2
                if part == 0:
                    state[chain] = psA.tile(
                        [P, NS], f32, name=f"qk{p}_{chain}", tag="vqk", bufs=2)
                ps = state[chain]
                wtv = wt[:].rearrange("p (d j m) -> p d j m", j=2, m=P)
                for d2 in range(2 * part, 2 * part + 2):
                    nc.tensor.matmul(
                        ps[:], wtv[:, d2],
                        jview(xt2[d2], S)[:, :, sl * NS:(sl + 1) * NS],
                        perf_mode=DR, start=(d2 == 0), stop=(d2 == D2 - 1))
                if part == 1:
                    nc.vector.tensor_scalar(
                        out=dst[:, sl * NS:(sl + 1) * NS], in0=ps[:],
                        scalar1=bc[:, p:p + 1], scalar2=None, op0=ALU.add)

            def make_qk_state(p):
                if p == 0:
                    wqt, wkt = wq0, wk0
                else:
                    wqt = pA.tile([P, D2 * 2 * P], f8, name=f"wq{p}",
                                  tag="wq", bufs=2)
                    nc.sync.dma_start(
                        wqt[:].rearrange("p (d j m) -> p d j m", j=2, m=P),
                        wq_d[p])
                    wkt = pA.tile([P, D2 * 2 * P], f8, name=f"wk{p}",
                                  tag="wk", bufs=2)
                    nc.sync.dma_start(
                        wkt[:].rearrange("p (d j m) -> p d j m", j=2, m=P),
                        wk_d[p])
                qt = pA.tile([P, S], bf16, name=f"qt{p}", tag="qt", bufs=2)
                kt = pA.tile([P, S], bf16, name=f"kt{p}", tag="kt", bufs=2)
                return {"ops": ((wqt, bqc, qt), (wkt, bkc, kt)),
                        "qt": qt, "kt": kt}

            LAG = 2
            qk_state = make_qk_state(0)
            for chain in range(4):
                for part in range(2):
                    emit_qk_chain_part(0, chain, part, qk_state)

            pending = None
            for p in range(NP_):
                if p == 2:
                    emit_w1_loads()
                qt, kt = qk_state["qt"], qk_state["kt"]
                next_state = make_qk_state(p + 1) if p + 1 < NP_ else None

                ctxU = pA.tile([P, S], f32, name=f"ctxU{p}", tag="ctxU",
                               bufs=2)
                den4 = pA.tile([4, NS], f32, name=f"den4_{p}", tag="den4",
                               bufs=2)

                def emit_scores(sl, t, expt):
                    ps = psA.tile([P, 2 * NS], f32, name=f"sc{t}_{sl}",
                                  tag="sc", bufs=2)
                    for h in range(2):
                        nc.tensor.matmul(
                            ps[:, h * NS:(h + 1) * NS],
                            kt[h * HD:(h + 1) * HD, t * P:(t + 1) * P],
                            qt[h * HD:(h + 1) * HD, sl * NS:(sl + 1) * NS],
                            start=True, stop=True,
                            tile_position=(h * HD, 0))
                    if t % 2 == 0:
                        expt[t // 2] = pExp.tile(
                            [P, 2 * 2 * NS], f8, name=f"e{t // 2}_{sl}",
                            tag="exp", bufs=3)
                    e2 = expt[t // 2]
                    nc.scalar.activation(jview(e2, 2 * NS)[:, t % 2, :],
                                         ps[:], AF.Exp, scale=e_scale,
                                         bias=ebias[:])

                def emit_ctx(sl, t2, cps, expt):
                    for h in range(2):
                        lhs = jview(v65[t2], H * 65)[
                            :, :, (2 * p + h) * 65:(2 * p + h) * 65 + 65]
                        nc.tensor.matmul(
                            cps[h][0:65, :], lhs,
                            jview(expt[t2], 2 * NS)[
                                :, :, h * NS:(h + 1) * NS],
                            perf_mode=DR,
                            start=(t2 == 0), stop=(t2 == T2 - 1))

                def emit_evict(sl, cps):
                    for h in range(2):
                        ps = cps[h]
                        stage = pA.tile([65, NS], f32, name=f"stg{h}{sl}",
                                        tag="rc", bufs=2)
                        nc.vector.tensor_copy(stage[64:65, :], ps[64:65, :])
                        nc.sync.dma_start(
                            den4[h * SL + sl:h * SL + sl + 1, :],
                            stage[64:65, :])
                        if h == 0:
                            nc.vector.tensor_copy(
                                ctxU[0:HD, sl * NS:(sl + 1) * NS],
                                ps[0:HD, :])
                        else:
                            tmp = pA.tile([HD, NS], f32, name=f"ctmp{sl}",
                                          tag="ctmp", bufs=2)
                            nc.vector.tensor_copy(tmp[:], ps[0:HD, :])
                            nc.sync.dma_start(
                                ctxU[HD:P, sl * NS:(sl + 1) * NS], tmp[:])

                expt0 = {}
                cps0 = [psA.tile([P, NS], f32, name=f"cps{h}_0", tag="ctx",
                                 bufs=2) for h in range(2)]
                expt1 = {}
                cps1 = [psA.tile([P, NS], f32, name=f"cps{h}_1", tag="ctx",
                                 bufs=2) for h in range(2)]
                if p == 0:
                    # A: scores(sl0) + the whole V projection interleaved
                    for t in range(ST):
                        emit_scores(0, t, expt0)
                        for hc in range(4 * t, 4 * t + 4):
                            emit_v_chunk(hc)
                    # B: scores(sl1) + lagged ctx(sl0) per token pair
                    for t in range(ST + LAG):
                        if t < ST:
                            emit_scores(1, t, expt1)
                        if t >= LAG and (t - LAG) % 2 == 1:
                            emit_ctx(0, (t - LAG) // 2, cps0, expt0)
                    emit_evict(0, cps0)
                    # C: ctx(sl1) + QK(1) chunks
                    for t in range(ST):
                        if t % 2 == 1:
                            emit_ctx(1, t // 2, cps1, expt1)
                        if next_state is not None:
                            emit_qk_chain_part(p + 1, t // 2, t % 2,
                                               next_state)
                    emit_evict(1, cps1)
                    pV_cm.__exit__(None, None, None)
                else:
                    # A: scores(sl0) + QK(p+1) chunks 0-3 + lagged ctx(sl0)
                    for t in range(ST + LAG):
                        if t < ST:
                            emit_scores(0, t, expt0)
                            if next_state is not None and t < 4:
                                emit_qk_chain_part(p + 1, t // 2, t % 2,
                                                   next_state)
                        if t >= LAG and (t - LAG) % 2 == 1:
                            emit_ctx(0, (t - LAG) // 2, cps0, expt0)
                    emit_evict(0, cps0)
                    if pending is not None:
                        emit_normalize(*pending)
                    # B: scores(sl1) + QK(p+1) chunks 4-7 + lagged ctx(sl1)
                    for t in range(ST + LAG):
                        if t < ST:
                            emit_scores(1, t, expt1)
                            if next_state is not None and t < 4:
                                emit_qk_chain_part(p + 1, (t + 4) // 2,
                                                   t % 2, next_state)
                        if t >= LAG and (t - LAG) % 2 == 1:
                            emit_ctx(1, (t - LAG) // 2, cps1, expt1)
                    emit_evict(1, cps1)
                pending = (p, ctxU, den4)
                qk_state = next_state
            emit_normalize(*pending)
            pExp_cm.__exit__(None, None, None)

        # ---------------- Wo + LN1 scope ----------------
        # hn = LN1(RSC*(x + bo) + ctxT2@wo2) * g1 + (be1 + b2); the RSC scale
        # on both residual and product cancels in LayerNorm.
        h_nat = []
        ht = [pH.tile([P, S], bf16, name=f"ht{d}", tag="ht", bufs=DT)
              for d in range(DT)]
        with tc.tile_pool(name="pWo", bufs=1) as pWo, \
             tc.tile_pool(name="psW", bufs=1, space="PSUM") as psW:
            bo_b = bcast_row(pWo, "bo", bo_d, D)
            g1_b = bcast_row(pWo, "g1", g1_d, D)
            be1_b = bcast_row(pWo, "be1", be1_d, D)

            # residual tiles: RSC*(x + bo), prefetched for all si
            xns = []
            for si in range(ST):
                xn = pWo.tile([P, D], f32, name=f"xn{si}", tag="xn", bufs=ST)
                nc.sync.dma_start(xn[:], x_d[si * P:(si + 1) * P, :])
                nc.vector.tensor_add(xn[:], xn[:], bo_b[:])
                nc.vector.tensor_scalar_mul(xn[:], xn[:], RSC)
                xns.append(xn)

            for si in range(ST):
                pss = [psW.tile([P, ND], f32, name=f"c{si}_{n}", tag="c",
                                bufs=4) for n in range(DL)]
                for q2 in range(D2):
                    for n in range(DL):
                        nc.tensor.matmul(
                            pss[n][:],
                            jview(ctxT2[q2], S)[:, :, si * P:(si + 1) * P],
                            jview(wo2[q2], D)[:, :, n * ND:(n + 1) * ND],
                            perf_mode=DR,
                            start=(q2 == 0), stop=(q2 == D2 - 1))
                st = pWo.tile([P, 8], f32, name=f"st1_{si}", tag="st1",
                              bufs=4)
                s1 = st[:, 2:3]
                s2 = st[:, 3:4]; mu = st[:, 4:5]; var = st[:, 5:6]
                rstd = st[:, 6:7]; nm = st[:, 7:8]
                v = pWo.tile([P, D], f32, name=f"v{si}", tag="v", bufs=4)
                scr = pWo.tile([P, D], f32, name=f"scr{si}", tag="scr",
                               bufs=3)
                for n in range(DL):
                    nc.vector.tensor_add(
                        v[:, n * ND:(n + 1) * ND], pss[n][:],
                        xns[si][:, n * ND:(n + 1) * ND])
                nc.scalar.activation(scr[:], v[:], AF.Copy, accum_out=s1)
                nc.scalar.activation(scr[:], v[:], AF.Square, accum_out=s2)
                nc.vector.tensor_scalar_mul(mu, s1, 1.0 / D)
                nc.vector.tensor_scalar_mul(var, s2, 1.0 / D)
                nc.vector.tensor_mul(nm, mu, mu)
                nc.vector.tensor_sub(var, var, nm)
                nc.vector.tensor_scalar_add(var, var, EPS * RSC * RSC)
                nc.scalar.sqrt(var, var)
                nc.vector.reciprocal(rstd, var)
                nc.vector.tensor_mul(nm, mu, rstd)
                nc.vector.tensor_scalar_mul(nm, nm, -1.0)
                hn = pH.tile([P, D], bf16, name=f"hn{si}", tag="hn", bufs=ST)
                nc.scalar.activation(scr[:], v[:], AF.Identity,
                                     bias=nm, scale=rstd)
                nc.vector.tensor_mul(scr[:], scr[:], g1_b[:])
                nc.vector.tensor_add(hn[:], scr[:], be1_b[:])
                h_nat.append(hn)
                # h^T transposes for this si; copybacks alternate DVE/ACT
                for dd in range(DT):
                    ps = psW.tile([P, P], bf16, name=f"tp{si}_{dd}", tag="tp",
                                  bufs=4)
                    nc.tensor.transpose(
                        ps[:], hn[:, dd * P:(dd + 1) * P], identity[:])
                    dst = ht[dd][:, si * P:(si + 1) * P]
                    if dd % 2 == 0:
                        nc.vector.tensor_copy(dst, ps[:])
                    else:
                        nc.scalar.copy(dst, ps[:])
        pCtx_cm.__exit__(None, None, None)

        # ---------------- FFN + LN2 scope ----------------
        with tc.tile_pool(name="pF", bufs=1) as pF:
            g2_b = bcast_row(pF, "g2", g2_d, D)
            be2_b = bcast_row(pF, "be2", be2_d, D)

            # W2 streamed (once) during the FFN1 window
            w2 = []
            for f in range(FT):
                t = pF.tile([P, D], bf16, name=f"w2_{f}", tag="w2", bufs=FT)
                nc.sync.dma_start(t[:], w2_d[f * P:(f + 1) * P, :])
                w2.append(t)

            ut = []
            with tc.tile_pool(name="psU", bufs=1, space="PSUM") as psU:
                for f in range(FT):
                    u = pF.tile([P, S], bf16, name=f"ut{f}", tag="ut",
                                bufs=FT)
                    for hf in range(SL):
                        ps = psU.tile([P, NS], f32, name=f"u{f}_{hf}",
                                      tag="u", bufs=4)
                        for d in range(DT):
                            nc.tensor.matmul(
                                ps[:], w1[f][:, d * P:(d + 1) * P],
                                ht[d][:, hf * NS:(hf + 1) * NS],
                                start=(d == 0), stop=(d == DT - 1))
                        nc.scalar.activation(u[:, hf * NS:(hf + 1) * NS],
                                             ps[:], AF.Relu,
                                             bias=b1c[:, f:f + 1])
                    ut.append(u)

            with tc.tile_pool(name="psY", bufs=1, space="PSUM") as psY:
                for si in range(ST):
                    pss = [psY.tile([P, ND], f32, name=f"y{si}_{n}", tag="y",
                                    bufs=4) for n in range(DL)]
                    for f in range(FT):
                        for n in range(DL):
                            nc.tensor.matmul(
                                pss[n][:],
                                ut[f][:, si * P:(si + 1) * P],
                                w2[f][:, n * ND:(n + 1) * ND],
                                start=(f == 0), stop=(f == FT - 1))
                    st = pF.tile([P, 8], f32, name=f"st2_{si}", tag="st2",
                                 bufs=4)
                    s1 = st[:, 2:3]
                    s2 = st[:, 3:4]; mu = st[:, 4:5]; var = st[:, 5:6]
                    rstd = st[:, 6:7]; nm = st[:, 7:8]
                    v = pF.tile([P, D], f32, name=f"v2_{si}", tag="v2",
                                bufs=4)
                    scr = pF.tile([P, D], f32, name=f"scr2_{si}", tag="scr2",
                                  bufs=3)
                    for n in range(DL):
                        nc.vector.tensor_add(
                            v[:, n * ND:(n + 1) * ND], pss[n][:],
                            h_nat[si][:, n * ND:(n + 1) * ND])
                    nc.scalar.activation(scr[:], v[:], AF.Copy, accum_out=s1)
                    nc.scalar.activation(scr[:], v[:], AF.Square,
                                         accum_out=s2)
                    nc.vector.tensor_scalar_mul(mu, s1, 1.0 / D)
                    nc.vector.tensor_scalar_mul(var, s2, 1.0 / D)
                    nc.vector.tensor_mul(nm, mu, mu)
                    nc.vector.tensor_sub(var, var, nm)
                    nc.vector.tensor_scalar_add(var, var, EPS)
                    nc.scalar.sqrt(var, var)
                    nc.vector.reciprocal(rstd, var)
                    nc.vector.tensor_mul(nm, mu, rstd)
                    nc.vector.tensor_scalar_mul(nm, nm, -1.0)
                    nc.scalar.activation(scr[:], v[:], AF.Identity,
                                         bias=nm, scale=rstd)
                    o = pF.tile([P, D], f32, name=f"o{si}", tag="o", bufs=3)
                    nc.vector.tensor_mul(scr[:], scr[:], g2_b[:])
                    nc.vector.tensor_add(o[:], scr[:], be2_b[:])
                    nc.sync.dma_start(out_d[si * P:(si + 1) * P, :], o[:])

    nc.compile()
    return nc


def pack_core_inputs(x_b, shared):
    """Per-core input map: batch element x_b + shared (prepacked) weights."""
    m = dict(shared)
    x_b = np.asarray(x_b, dtype=np.float32)
    m["x"] = np.ascontiguousarray(x_b)
    # x^T in fp8, dv-pair interleaved: [D2, P, 2, S]
    xT = np.ascontiguousarray(x_b.T)
    m["xT"] = np.ascontiguousarray(
        xT.reshape(D2, 2, P, S).transpose(0, 2, 1, 3).astype(np_f8))
    return m


def pack_shared(Wq, bq, Wk, bk, Wv, bv, Wo, bo, ln1_g, ln1_b, W1, b1, W2, b2,
                ln2_g, ln2_b):
    """Host-side layout packing of the replicated weights. fp8 weights are
    prescaled by WSC=32 (their sigma is ~1/32); be1 absorbs b2 exactly with
    b1 -= W1^T b2 compensating FFN1."""
    f = np.float32
    Wq = np.asarray(Wq, dtype=f); Wk = np.asarray(Wk, dtype=f)
    Wv = np.asarray(Wv, dtype=f); Wo = np.asarray(Wo, dtype=f)
    W1 = np.asarray(W1, dtype=f); W2 = np.asarray(W2, dtype=f)
    b1 = np.asarray(b1, dtype=f); b2 = np.asarray(b2, dtype=f)
    ln1_b = np.asarray(ln1_b, dtype=f)
    # [pair, p, d2, j, m]: DoubleRow dv-pair layout, contiguous per-pair DMA
    pack_qk = lambda W: np.ascontiguousarray(
        (WSC * W).reshape(D, H * HD).reshape(D2, 2, P, NP_, P).transpose(
            3, 2, 0, 1, 4).astype(np_f8))
    pack_dv = lambda W: np.ascontiguousarray(
        (WSC * W).reshape(D2, 2, P, D).transpose(0, 2, 1, 3).astype(np_f8))
    sel = np.zeros((SL, 4, P), dtype=f)
    for sl in range(SL):
        for m in range(P):
            sel[sl, (m // HD) * SL + sl, m] = 1.0
    be1_f = ln1_b + b2
    b1_f = (b1.astype(np.float64) -
            W1.astype(np.float64).T @ b2.astype(np.float64)).astype(f)
    return {
        "vones": np.ones((P, H), dtype=np_f8),
        "sel": sel,
        "Wq": pack_qk(Wq), "Wk": pack_qk(Wk),
        "Wv": pack_dv(Wv.reshape(D, D)),
        "Wo": pack_dv(Wo),
        "W1": np.ascontiguousarray(
            W1.reshape(DT, P, FT, P).transpose(2, 1, 0, 3)).astype(np_bf16),
        "W2": np.ascontiguousarray(W2).astype(np_bf16),
        "bqc": np.ascontiguousarray(WSC * np.asarray(bq, f).reshape(NP_, P).T),
        "bkc": np.ascontiguousarray(WSC * np.asarray(bk, f).reshape(NP_, P).T),
        "b1c": np.ascontiguousarray(b1_f.reshape(FT, P).T),
        "bv": np.ascontiguousarray(WSC * np.asarray(bv, f).reshape(D)),
        "bo": np.ascontiguousarray(bo, dtype=f),
        "g1": np.ascontiguousarray(ln1_g, dtype=f),
        "be1": np.ascontiguousarray(be1_f),
        "g2": np.ascontiguousarray(ln2_g, dtype=f),
        "be2": np.ascontiguousarray(ln2_b, dtype=f),
    }


_NC_CACHE = {}


def get_nc():
    if "nc" not in _NC_CACHE:
        _NC_CACHE["nc"] = build_encoder(num_devices=8)
    return _NC_CACHE["nc"]


def kernel(x, Wq, bq, Wk, bk, Wv, bv, Wo, bo, ln1_g, ln1_b, W1, b1, W2, b2,
           ln2_g, ln2_b):
    x = np.asarray(x)
    assert x.shape == (B, S, D)
    shared = pack_shared(Wq, bq, Wk, bk, Wv, bv, Wo, bo, ln1_g, ln1_b,
                         W1, b1, W2, b2, ln2_g, ln2_b)
    in_maps = [pack_core_inputs(x[b], shared) for b in range(B)]
    nc = get_nc()
    res = bass_utils.run_bass_kernel_spmd(
        nc, in_maps, core_ids=list(range(B)), trace=False)
    return np.stack([res.results[b]["out"] for b in range(B)], axis=0)


# revision 22
# speedup vs baseline: 1.0630x; 1.0630x over previous
"""Transformer encoder layer (nn_Encoder) on 8 TRN2 NeuronCores.

Strategy: data-parallel over batch — B=8, one batch element per core, weights
replicated, no collectives. Per core a single Bass/Tile kernel computes the
whole layer.

Precision plan: the attention path (QKV projections, softmax-weighted context,
Wo) runs in fp8e4m3 with DoubleRow matmuls (2 contraction tiles per
instruction, 0.5 cycles/row) — its error is attenuated ~25x by the residual
(|attn_out| ~ 0.04 |x|). Attention scores stay bf16. The FFN (whose output is
NOT small vs the residual) runs in bf16 at 1 cycle/row. All accumulation in
fp32 PSUM. fp8 operands are host-prescaled by 32 (weights are sigma~1/32 and
would hit the fp8 denormal floor); the combined 1024x scale on the attention
output is folded into the residual (x side is scaled 1024x instead) and
neutralized by LayerNorm's scale invariance.

Layout: attention runs in the "transposed domain" ([feature, tokens]) so every
weight matmul uses natural weight layouts; softmax over tokens-on-partitions is
handled by appending a ones-column to V (denominator lands in the ctx matmul's
extra output row, M=65). Per pair the 4 denominator rows are staged into one
[4, NS] tile, inverted with a single reciprocal, broadcast across partitions
with a tiny K=4 matmul against a selection matrix, and applied in one
full-width multiply per slice (deferred one pair to keep PE fed). Wo/FFN2
products land in the natural domain where both LayerNorms reduce along the
free dim; the normalize runs on ACT via per-partition scale/bias. b2 is folded
into be1 on the host (with b1 -= W1^T b2 compensating FFN1). W1/Wo are
prefetched into SBUF during the attention phase and W2 during FFN1, each read
from HBM exactly once. FFN2 runs si-major with per-si epilogues so only the
last tile's epilogue trails the final matmul.

Self-contained: hardcodes B=8, S=1024, D=1024, H=16, FF=2048, 8 cores.
"""
import math
import numpy as np
import ml_dtypes
from contextlib import ExitStack

import concourse.bass as bass
import concourse.tile as tile
from concourse import bacc, mybir
from concourse import bass_utils
from concourse.masks import make_identity

B = 8
S = 1024
D = 1024
H = 16
FF = 2048
P = 128
HD = 64
EPS = 1e-5
f32 = mybir.dt.float32
f32r = mybir.dt.float32r
bf16 = mybir.dt.bfloat16
f8 = mybir.dt.float8e4
np_bf16 = ml_dtypes.bfloat16
np_f8 = ml_dtypes.float8_e4m3
DR = mybir.MatmulPerfMode.DoubleRow
AF = mybir.ActivationFunctionType
ALU = mybir.AluOpType

NP_ = H // 2          # head pairs
ST = S // P           # token tiles
DT = D // P
D2 = DT // 2          # contraction pair-tiles over D
T2 = ST // 2          # token pair-tiles
FT = FF // P
NS = 512              # token slice width (matmul free dim)
SL = S // NS
ND = 512              # feature slice width
DL = D // ND

WSC = 32.0            # fp8 weight prescale (weights are sigma ~ 1/32)
RSC = WSC * WSC       # resulting scale on the attention output


def build_encoder(num_devices=8):
    # q,k arrive prescaled by WSC each: scores carry WSC^2; exp folds it into
    # its input scale. The extra -2ln2 bias keeps exp outputs < 240/4 (fp8e4m3
    # max is 240); softmax cancels any constant factor on e.
    e_scale = 1.0 / (math.sqrt(HD) * RSC)
    e_bias = -2.0 * math.log(2.0)
    nc = bacc.Bacc("TRN2", target_bir_lowering=False, debug=False,
                   enable_asserts=True, num_devices=num_devices)

    dram = lambda n, sh, dt: nc.dram_tensor(n, sh, dt, kind="ExternalInput").ap()
    xT_d = dram("xT", [D2, P, 2, S], f8)
    vones_d = dram("vones", [P, H], f8)
    sel_d = dram("sel", [SL, 4, P], f32r)
    x_d = dram("x", [S, D], f32)
    wq_d = dram("Wq", [NP_, P, D2, 2, P], f8)
    wk_d = dram("Wk", [NP_, P, D2, 2, P], f8)
    wv_d = dram("Wv", [D2, P, 2, D], f8)
    wo_d = dram("Wo", [D2, P, 2, D], f8)
    w1_d = dram("W1", [FT, P, DT, P], bf16)
    w2_d = dram("W2", [FF, D], bf16)
    bqc_d = dram("bqc", [P, NP_], f32)
    bkc_d = dram("bkc", [P, NP_], f32)
    b1c_d = dram("b1c", [P, FT], f32)
    bv_d = dram("bv", [D], f32)
    bo_d = dram("bo", [D], f32)
    g1_d = dram("g1", [D], f32)
    be1_d = dram("be1", [D], f32)
    g2_d = dram("g2", [D], f32)
    be2_d = dram("be2", [D], f32)
    out_d = nc.dram_tensor("out", [S, D], f32, kind="ExternalOutput").ap()

    with tile.TileContext(nc) as tc, ExitStack() as octx:
        const = octx.enter_context(tc.tile_pool(name="const", bufs=1))
        identity = const.tile([P, P], bf16, name="identity")
        make_identity(nc, identity)
        ebias = const.tile([P, 1], f32, name="ebias")
        nc.gpsimd.memset(ebias[:], e_bias)

        # long-lived weight pool (prefetched during attention) + h tiles
        pW = octx.enter_context(tc.tile_pool(name="pW", bufs=1))
        pH = octx.enter_context(tc.tile_pool(name="pH", bufs=1))

        def bcast_row(pool, name, src_row, width, tag=None):
            r = pool.tile([1, width], f32, name=f"{name}_r", tag="bcr", bufs=1)
            nc.sync.dma_start(r[:], src_row[None, :])
            b = pool.tile([P, width], f32, name=f"{name}_b",
                          tag=(tag or f"{name}_b"))
            nc.gpsimd.partition_broadcast(b[:], r[:])
            return b

        jview = lambda ap, w: ap.rearrange("p (j c) -> p j c", j=2)

        # ctxT pool (attention -> Wo), fp8 dv-pair tiles [P, 2, S]
        pCtx_cm = tc.tile_pool(name="pCtx", bufs=1)
        pCtx = pCtx_cm.__enter__()

        # ---------------- attention scope ----------------
        with tc.tile_pool(name="pA", bufs=1) as pA, \
             tc.tile_pool(name="psA", bufs=1, space="PSUM") as psA:

            # pair-0 Q/K weights first so QK(0) matmuls start ASAP
            wq0 = pA.tile([P, D2 * 2 * P], f8, name="wq0", tag="wq", bufs=2)
            nc.sync.dma_start(
                wq0[:].rearrange("p (d j m) -> p d j m", j=2, m=P), wq_d[0])
            wk0 = pA.tile([P, D2 * 2 * P], f8, name="wk0", tag="wk", bufs=2)
            nc.sync.dma_start(
                wk0[:].rearrange("p (d j m) -> p d j m", j=2, m=P), wk_d[0])

            # x^T fp8 pair tiles [P, 2, S]
            xt2 = []
            for d2 in range(D2):
                t = pA.tile([P, 2 * S], f8, name=f"xt{d2}", tag="xt", bufs=D2)
                nc.sync.dma_start(jview(t, S), xT_d[d2])
                xt2.append(t)

            # ---- V projection weights ----
            pExp_cm = tc.tile_pool(name="pExp", bufs=1)
            pExp = pExp_cm.__enter__()
            pV_cm = tc.tile_pool(name="pV", bufs=1)
            pV = pV_cm.__enter__()
            wv2 = []
            for d2 in range(D2):
                t = pV.tile([P, 2 * D], f8, name=f"wv{d2}", tag="wv", bufs=D2)
                nc.sync.dma_start(jview(t, D), wv_d[d2])
                wv2.append(t)

            # V65 token-pair tiles: [128, 2, H*65], ones at [:, :, 65h+64]
            v65 = []
            for t2 in range(T2):
                v = pA.tile([P, 2 * H * 65], f8, name=f"v65_{t2}", tag="v65",
                            bufs=T2)
                nc.gpsimd.memset(
                    jview(v, H * 65).rearrange(
                        "p j (h c) -> p j h c", c=65)[:, :, :, 64:65], 1.0)
                v65.append(v)

            # small consts (needed only after the first QK chains land)
            bqc = const.tile([P, NP_], f32, name="bqc")
            nc.sync.dma_start(bqc[:], bqc_d)
            bkc = const.tile([P, NP_], f32, name="bkc")
            nc.sync.dma_start(bkc[:], bkc_d)
            b1c = const.tile([P, FT], f32, name="b1c")
            nc.sync.dma_start(b1c[:], b1c_d)
            selt = const.tile([4, SL * P], f32r, name="selt")
            for sl in range(SL):
                nc.sync.dma_start(selt[:, sl * P:(sl + 1) * P], sel_d[sl])
            bv_b = bcast_row(pA, "bv", bv_d, D)

            # Wo prefetch (fp8 dv-pair tiles [P, 2, D])
            wo2 = []
            for q2 in range(D2):
                t = pW.tile([P, 2 * D], f8, name=f"wo{q2}", tag="wo", bufs=D2)
                nc.sync.dma_start(jview(t, D), wo_d[q2])
                wo2.append(t)

            # W1/W2 resident tiles; DMAs emitted later in the pair loop
            w1 = [pW.tile([P, DT * P], bf16, name=f"w1_{f}", tag="w1", bufs=FT)
                  for f in range(FT)]
            w2 = [pW.tile([P, D], bf16, name=f"w2_{f}", tag="w2", bufs=FT)
                  for f in range(FT)]

            def emit_w1_loads():
                for f in range(FT):
                    nc.sync.dma_start(
                        w1[f][:].rearrange("p (dt q) -> p dt q", q=P),
                        w1_d[f])

            def emit_w2_loads():
                for f in range(FT):
                    nc.sync.dma_start(w2[f][:], w2_d[f * P:(f + 1) * P, :])

            hpn = ND // HD
            v_state = {}

            def emit_v_chunk(hc):
                """Half-chunk hc of the V projection (chain = hc//2)."""
                chain = hc // 2
                part = hc % 2
                t, n = chain // DL, chain % DL
                if part == 0:
                    v_state[chain] = psA.tile(
                        [P, ND], f32, name=f"vps{t}_{n}", tag="vqk", bufs=2)
                ps = v_state[chain]
                for d2 in range(2 * part, 2 * part + 2):
                    nc.tensor.matmul(
                        ps[:], jview(xt2[d2], S)[:, :, t * P:(t + 1) * P],
                        jview(wv2[d2], D)[:, :, n * ND:(n + 1) * ND],
                        perf_mode=DR, start=(d2 == 0), stop=(d2 == D2 - 1))
                if part == 1:
                    dst = jview(v65[t // 2], H * 65).rearrange(
                        "p j (h c) -> p j h c", c=65)[
                        :, t % 2, n * hpn:(n + 1) * hpn, 0:64]
                    srcv = ps[:].rearrange("p (h k) -> p h k", k=HD)
                    bvs = bv_b[:, n * ND:(n + 1) * ND].rearrange(
                        "p (h k) -> p h k", k=HD)
                    nc.vector.tensor_add(dst, srcv, bvs)

            # ---- attention per head pair ----
            ctxT2 = [pCtx.tile([P, 2 * S], f8, name=f"ctxT{q2}", tag="ctxT",
                               bufs=D2) for q2 in range(D2)]

            def emit_normalize(p, ctxU, den4):
                """Deferred softmax-normalize of pair p's ctx."""
                den4r = pA.tile([4, NS], f32r, name=f"den4r_{p}", tag="den4r",
                                bufs=2)
                with nc.allow_low_precision("softmax denom recip in f32r"):
                    nc.vector.reciprocal(den4r[:], den4[:])
                for sl in range(SL):
                    rcb = psA.tile([P, NS], f32, name=f"rcb{p}_{sl}",
                                   tag="vqk", bufs=2)
                    nc.tensor.matmul(rcb[:], selt[:, sl * P:(sl + 1) * P],
                                     den4r[:], start=True, stop=True)
                    nc.vector.tensor_mul(
                        jview(ctxT2[p // 2], S)[
                            :, p % 2, sl * NS:(sl + 1) * NS],
                        ctxU[:, sl * NS:(sl + 1) * NS], rcb[:])

            def emit_qk_chain_part(p, chain, part, state):
                """Emit 2 of the 4 DoubleRow accumulation matmuls of QK chain
                (chain: 0..3 = Q-sl0, Q-sl1, K-sl0, K-sl1) for pair p."""
                wt, bc, dst = state["ops"][chain // 2]
                sl = chain % 2
                if part == 0:
                    state[chain] = psA.tile(
                        [P, NS], f32, name=f"qk{p}_{chain}", tag="vqk", bufs=2)
                ps = state[chain]
                wtv = wt[:].rearrange("p (d j m) -> p d j m", j=2, m=P)
                for d2 in range(2 * part, 2 * part + 2):
                    nc.tensor.matmul(
                        ps[:], wtv[:, d2],
                        jview(xt2[d2], S)[:, :, sl * NS:(sl + 1) * NS],
                        perf_mode=DR, start=(d2 == 0), stop=(d2 == D2 - 1))
                if part == 1:
                    nc.vector.tensor_scalar(
                        out=dst[:, sl * NS:(sl + 1) * NS], in0=ps[:],
                        scalar1=bc[:, p:p + 1], scalar2=None, op0=ALU.add)

            def make_qk_state(p):
                if p == 0:
                    wqt, wkt = wq0, wk0
                else:
                    wqt = pA.tile([P, D2 * 2 * P], f8, name=f"wq{p}",
                                  tag="wq", bufs=2)
                    nc.sync.dma_start(
                        wqt[:].rearrange("p (d j m) -> p d j m", j=2, m=P),
                        wq_d[p])
                    wkt = pA.tile([P, D2 * 2 * P], f8, name=f"wk{p}",
                                  tag="wk", bufs=2)
                    nc.sync.dma_start(
                        wkt[:].rearrange("p (d j m) -> p d j m", j=2, m=P),
                        wk_d[p])
                qt = pA.tile([P, S], bf16, name=f"qt{p}", tag="qt", bufs=2)
                kt = pA.tile([P, S], bf16, name=f"kt{p}", tag="kt", bufs=2)
                return {"ops": ((wqt, bqc, qt), (wkt, bkc, kt)),
                        "qt": qt, "kt": kt}

            LAG = 2
            qk_state = make_qk_state(0)
            for chain in range(4):
                for part in range(2):
                    emit_qk_chain_part(0, chain, part, qk_state)

            pending = None
            for p in range(NP_):
                if p == 2:
                    emit_w1_loads()
                if p == 5:
                    emit_w2_loads()
                qt, kt = qk_state["qt"], qk_state["kt"]
                next_state = make_qk_state(p + 1) if p + 1 < NP_ else None

                ctxU = pA.tile([P, S], f32, name=f"ctxU{p}", tag="ctxU",
                               bufs=2)
                den4 = pA.tile([4, NS], f32, name=f"den4_{p}", tag="den4",
                               bufs=2)

                def emit_scores(sl, t, expt):
                    ps = psA.tile([P, 2 * NS], f32, name=f"sc{t}_{sl}",
                                  tag="sc", bufs=2)
                    for h in range(2):
                        nc.tensor.matmul(
                            ps[:, h * NS:(h + 1) * NS],
                            kt[h * HD:(h + 1) * HD, t * P:(t + 1) * P],
                            qt[h * HD:(h + 1) * HD, sl * NS:(sl + 1) * NS],
                            start=True, stop=True,
                            tile_position=(h * HD, 0))
                    if t % 2 == 0:
                        expt[t // 2] = pExp.tile(
                            [P, 2 * 2 * NS], f8, name=f"e{t // 2}_{sl}",
                            tag="exp", bufs=3)
                    e2 = expt[t // 2]
                    nc.scalar.activation(jview(e2, 2 * NS)[:, t % 2, :],
                                         ps[:], AF.Exp, scale=e_scale,
                                         bias=ebias[:])

                def emit_ctx(sl, t2, cps, expt):
                    for h in range(2):
                        lhs = jview(v65[t2], H * 65)[
                            :, :, (2 * p + h) * 65:(2 * p + h) * 65 + 65]
                        nc.tensor.matmul(
                            cps[h][0:65, :], lhs,
                            jview(expt[t2], 2 * NS)[
                                :, :, h * NS:(h + 1) * NS],
                            perf_mode=DR,
                            start=(t2 == 0), stop=(t2 == T2 - 1))

                def emit_evict(sl, cps):
                    for h in range(2):
                        ps = cps[h]
                        stage = pA.tile([65, NS], f32, name=f"stg{h}{sl}",
                                        tag="rc", bufs=2)
                        nc.vector.tensor_copy(stage[64:65, :], ps[64:65, :])
                        nc.sync.dma_start(
                            den4[h * SL + sl:h * SL + sl + 1, :],
                            stage[64:65, :])
                        if h == 0:
                            nc.vector.tensor_copy(
                                ctxU[0:HD, sl * NS:(sl + 1) * NS],
                                ps[0:HD, :])
                        else:
                            tmp = pA.tile([HD, NS], f32, name=f"ctmp{sl}",
                                          tag="ctmp", bufs=2)
                            nc.vector.tensor_copy(tmp[:], ps[0:HD, :])
                            nc.sync.dma_start(
                                ctxU[HD:P, sl * NS:(sl + 1) * NS], tmp[:])

                expt0 = {}
                cps0 = [psA.tile([P, NS], f32, name=f"cps{h}_0", tag="ctx",
                                 bufs=2) for h in range(2)]
                expt1 = {}
                cps1 = [psA.tile([P, NS], f32, name=f"cps{h}_1", tag="ctx",
                                 bufs=2) for h in range(2)]
                if p == 0:
                    # A: scores(sl0) + the whole V projection interleaved
                    for t in range(ST):
                        emit_scores(0, t, expt0)
                        for hc in range(4 * t, 4 * t + 4):
                            emit_v_chunk(hc)
                    # B: scores(sl1) + lagged ctx(sl0) per token pair
                    for t in range(ST + LAG):
                        if t < ST:
                            emit_scores(1, t, expt1)
                        if t >= LAG and (t - LAG) % 2 == 1:
                            emit_ctx(0, (t - LAG) // 2, cps0, expt0)
                    emit_evict(0, cps0)
                    # C: ctx(sl1) + QK(1) chunks
                    for t in range(ST):
                        if t % 2 == 1:
                            emit_ctx(1, t // 2, cps1, expt1)
                        if next_state is not None:
                            emit_qk_chain_part(p + 1, t // 2, t % 2,
                                               next_state)
                    emit_evict(1, cps1)
                    pV_cm.__exit__(None, None, None)
                else:
                    # A: scores(sl0) + QK(p+1) chunks 0-3 + lagged ctx(sl0)
                    for t in range(ST + LAG):
                        if t < ST:
                            emit_scores(0, t, expt0)
                            if next_state is not None and t < 4:
                                emit_qk_chain_part(p + 1, t // 2, t % 2,
                                                   next_state)
                        if t >= LAG and (t - LAG) % 2 == 1:
                            emit_ctx(0, (t - LAG) // 2, cps0, expt0)
                    emit_evict(0, cps0)
                    if pending is not None:
                        emit_normalize(*pending)
                    # B: scores(sl1) + QK(p+1) chunks 4-7 + lagged ctx(sl1)
                    for t in range(ST + LAG):
                        if t < ST:
                            emit_scores(1, t, expt1)
                            if next_state is not None and t < 4:
                                emit_qk_chain_part(p + 1, (t + 4) // 2,
                                                   t % 2, next_state)
                        if t >= LAG and (t - LAG) % 2 == 1:
                            emit_ctx(1, (t - LAG) // 2, cps1, expt1)
                    emit_evict(1, cps1)
                pending = (p, ctxU, den4)
                qk_state = next_state
            emit_normalize(*pending)
            pExp_cm.__exit__(None, None, None)

        # -------- Wo + LN1 + FFN merged scope (PE never starves) --------
        # hn = LN1(RSC*(x + bo) + ctxT2@wo2) * g1 + (be1 + b2); the RSC scale
        # on both residual and product cancels in LayerNorm. FFN1 half-0
        # chains are interleaved under the LN1/transpose epilogues of si 4-7
        # (the fp8 Wo matmuls alone cannot keep the PE fed).
        h_nat = []
        ht = [pH.tile([P, S], bf16, name=f"ht{d}", tag="ht", bufs=DT)
              for d in range(DT)]
        with tc.tile_pool(name="pWo", bufs=1) as pWo:
            psW_cm = tc.tile_pool(name="psW", bufs=1, space="PSUM")
            psW = psW_cm.__enter__()
            psU_cm = tc.tile_pool(name="psU", bufs=1, space="PSUM")
            psU = psU_cm.__enter__()
            bo_b = bcast_row(pWo, "bo", bo_d, D, tag="bc0")
            g1_b = bcast_row(pWo, "g1", g1_d, D, tag="bc1")
            be1_b = bcast_row(pWo, "be1", be1_d, D, tag="bc2")

            # residual tiles: RSC*(x + bo)
            xns = []
            for si in range(ST):
                xn = pWo.tile([P, D], f32, name=f"xn{si}", tag="xn", bufs=3)
                nc.sync.dma_start(xn[:], x_d[si * P:(si + 1) * P, :])
                nc.vector.tensor_add(xn[:], xn[:], bo_b[:])
                nc.vector.tensor_scalar_mul(xn[:], xn[:], RSC)
                xns.append(xn)

            ut = [pWo.tile([P, S], bf16, name=f"ut{f}", tag="ut", bufs=FT)
                  for f in range(FT)]

            def emit_ffn1(f, hf):
                ps = psU.tile([P, NS], f32, name=f"u{f}_{hf}", tag="u",
                              bufs=4)
                for d in range(DT):
                    nc.tensor.matmul(
                        ps[:], w1[f][:, d * P:(d + 1) * P],
                        ht[d][:, hf * NS:(hf + 1) * NS],
                        start=(d == 0), stop=(d == DT - 1))
                nc.scalar.activation(ut[f][:, hf * NS:(hf + 1) * NS],
                                     ps[:], AF.Relu, bias=b1c[:, f:f + 1])

            def ln_stats(pool, si, v, pfx):
                st = pool.tile([P, 8], f32, name=f"{pfx}st{si}", tag="st",
                               bufs=4)
                s1 = st[:, 2:3]
                s2 = st[:, 3:4]; mu = st[:, 4:5]; var = st[:, 5:6]
                rstd = st[:, 6:7]; nm = st[:, 7:8]
                scr = pool.tile([P, D], f32, name=f"{pfx}scr{si}", tag="scr",
                                bufs=2)
                nc.scalar.activation(scr[:], v[:], AF.Copy, accum_out=s1)
                nc.scalar.activation(scr[:], v[:], AF.Square, accum_out=s2)
                nc.vector.tensor_scalar_mul(mu, s1, 1.0 / D)
                nc.vector.tensor_scalar_mul(var, s2, 1.0 / D)
                nc.vector.tensor_mul(nm, mu, mu)
                nc.vector.tensor_sub(var, var, nm)
                nc.vector.tensor_scalar_add(var, var, EPS)
                nc.scalar.sqrt(var, var)
                nc.vector.reciprocal(rstd, var)
                nc.vector.tensor_mul(nm, mu, rstd)
                nc.vector.tensor_scalar_mul(nm, nm, -1.0)
                nc.scalar.activation(scr[:], v[:], AF.Identity,
                                     bias=nm, scale=rstd)
                return scr

            def emit_wo_si(si):
                pss = [psW.tile([P, ND], f32, name=f"c{si}_{n}", tag="c",
                                bufs=2) for n in range(DL)]
                for q2 in range(D2):
                    for n in range(DL):
                        nc.tensor.matmul(
                            pss[n][:],
                            jview(ctxT2[q2], S)[:, :, si * P:(si + 1) * P],
                            jview(wo2[q2], D)[:, :, n * ND:(n + 1) * ND],
                            perf_mode=DR,
                            start=(q2 == 0), stop=(q2 == D2 - 1))
                v = pWo.tile([P, D], f32, name=f"v{si}", tag="v", bufs=2)
                for n in range(DL):
                    nc.vector.tensor_add(
                        v[:, n * ND:(n + 1) * ND], pss[n][:],
                        xns[si][:, n * ND:(n + 1) * ND])
                scr = ln_stats(pWo, si, v, "ln1")
                hn = pH.tile([P, D], bf16, name=f"hn{si}", tag="hn", bufs=ST)
                nc.vector.tensor_mul(scr[:], scr[:], g1_b[:])
                nc.vector.tensor_add(hn[:], scr[:], be1_b[:])
                h_nat.append(hn)
                # h^T transposes for this si; copybacks alternate DVE/ACT
                for dd in range(DT):
                    ps = psW.tile([P, P], bf16, name=f"tp{si}_{dd}", tag="tp",
                                  bufs=2)
                    nc.tensor.transpose(
                        ps[:], hn[:, dd * P:(dd + 1) * P], identity[:])
                    dst = ht[dd][:, si * P:(si + 1) * P]
                    if dd % 2 == 0:
                        nc.vector.tensor_copy(dst, ps[:])
                    else:
                        nc.scalar.copy(dst, ps[:])

            for si in range(ST):
                emit_wo_si(si)
                if 3 <= si <= 6:
                    for f in range(4 * (si - 3), 4 * (si - 3) + 4):
                        emit_ffn1(f, 0)

            g2_b = bcast_row(pWo, "g2", g2_d, D, tag="bc0")
            be2_b = bcast_row(pWo, "be2", be2_d, D, tag="bc1")
            for f in range(FT):
                emit_ffn1(f, 1)
            psU_cm.__exit__(None, None, None)
            psW_cm.__exit__(None, None, None)

            # ---- FFN2 si-major with per-si LN2 epilogue ----
            with tc.tile_pool(name="psY", bufs=1, space="PSUM") as psY:
                for si in range(ST):
                    pss = [psY.tile([P, ND], f32, name=f"y{si}_{n}", tag="y",
                                    bufs=4) for n in range(DL)]
                    for f in range(FT):
                        for n in range(DL):
                            nc.tensor.matmul(
                                pss[n][:],
                                ut[f][:, si * P:(si + 1) * P],
                                w2[f][:, n * ND:(n + 1) * ND],
                                start=(f == 0), stop=(f == FT - 1))
                    v = pWo.tile([P, D], f32, name=f"v2_{si}", tag="v",
                                 bufs=2)
                    for n in range(DL):
                        nc.vector.tensor_add(
                            v[:, n * ND:(n + 1) * ND], pss[n][:],
                            h_nat[si][:, n * ND:(n + 1) * ND])
                    scr = ln_stats(pWo, si, v, "ln2")
                    o = pWo.tile([P, D], f32, name=f"o{si}", tag="o", bufs=2)
                    nc.vector.tensor_mul(scr[:], scr[:], g2_b[:])
                    nc.vector.tensor_add(o[:], scr[:], be2_b[:])
                    nc.sync.dma_start(out_d[si * P:(si + 1) * P, :], o[:])
        pCtx_cm.__exit__(None, None, None)

    nc.compile()
    return nc


def pack_core_inputs(x_b, shared):
    """Per-core input map: batch element x_b + shared (prepacked) weights."""
    m = dict(shared)
    x_b = np.asarray(x_b, dtype=np.float32)
    m["x"] = np.ascontiguousarray(x_b)
    # x^T in fp8, dv-pair interleaved: [D2, P, 2, S]
    xT = np.ascontiguousarray(x_b.T)
    m["xT"] = np.ascontiguousarray(
        xT.reshape(D2, 2, P, S).transpose(0, 2, 1, 3).astype(np_f8))
    return m


def pack_shared(Wq, bq, Wk, bk, Wv, bv, Wo, bo, ln1_g, ln1_b, W1, b1, W2, b2,
                ln2_g, ln2_b):
    """Host-side layout packing of the replicated weights. fp8 weights are
    prescaled by WSC=32 (their sigma is ~1/32); be1 absorbs b2 exactly with
    b1 -= W1^T b2 compensating FFN1."""
    f = np.float32
    Wq = np.asarray(Wq, dtype=f); Wk = np.asarray(Wk, dtype=f)
    Wv = np.asarray(Wv, dtype=f); Wo = np.asarray(Wo, dtype=f)
    W1 = np.asarray(W1, dtype=f); W2 = np.asarray(W2, dtype=f)
    b1 = np.asarray(b1, dtype=f); b2 = np.asarray(b2, dtype=f)
    ln1_b = np.asarray(ln1_b, dtype=f)
    # [pair, p, d2, j, m]: DoubleRow dv-pair layout, contiguous per-pair DMA
    pack_qk = lambda W: np.ascontiguousarray(
        (WSC * W).reshape(D, H * HD).reshape(D2, 2, P, NP_, P).transpose(
            3, 2, 0, 1, 4).astype(np_f8))
    pack_dv = lambda W: np.ascontiguousarray(
        (WSC * W).reshape(D2, 2, P, D).transpose(0, 2, 1, 3).astype(np_f8))
    sel = np.zeros((SL, 4, P), dtype=f)
    for sl in range(SL):
        for m in range(P):
            sel[sl, (m // HD) * SL + sl, m] = 1.0
    be1_f = ln1_b + b2
    b1_f = (b1.astype(np.float64) -
            W1.astype(np.float64).T @ b2.astype(np.float64)).astype(f)
    return {
        "vones": np.ones((P, H), dtype=np_f8),
        "sel": sel,
        "Wq": pack_qk(Wq), "Wk": pack_qk(Wk),
        "Wv": pack_dv(Wv.reshape(D, D)),
        "Wo": pack_dv(Wo),
        "W1": np.ascontiguousarray(
            W1.reshape(DT, P, FT, P).transpose(2, 1, 0, 3)).astype(np_bf16),
        "W2": np.ascontiguousarray(W2).astype(np_bf16),
        "bqc": np.ascontiguousarray(WSC * np.asarray(bq, f).reshape(NP_, P).T),
        "bkc": np.ascontiguousarray(WSC * np.asarray(bk, f).reshape(NP_, P).T),
        "b1c": np.ascontiguousarray(b1_f.reshape(FT, P).T),
        "bv": np.ascontiguousarray(WSC * np.asarray(bv, f).reshape(D)),
        "bo": np.ascontiguousarray(bo, dtype=f),
        "g1": np.ascontiguousarray(ln1_g, dtype=f),
        "be1": np.ascontiguousarray(be1_f),
        "g2": np.ascontiguousarray(ln2_g, dtype=f),
        "be2": np.ascontiguousarray(ln2_b, dtype=f),
    }


_NC_CACHE = {}


def get_nc():
    if "nc" not in _NC_CACHE:
        _NC_CACHE["nc"] = build_encoder(num_devices=8)
    return _NC_CACHE["nc"]


def kernel(x, Wq, bq, Wk, bk, Wv, bv, Wo, bo, ln1_g, ln1_b, W1, b1, W2, b2,
           ln2_g, ln2_b):
    x = np.asarray(x)
    assert x.shape == (B, S, D)
    shared = pack_shared(Wq, bq, Wk, bk, Wv, bv, Wo, bo, ln1_g, ln1_b,
                         W1, b1, W2, b2, ln2_g, ln2_b)
    in_maps = [pack_core_inputs(x[b], shared) for b in range(B)]
    nc = get_nc()
    res = bass_utils.run_bass_kernel_spmd(
        nc, in_maps, core_ids=list(range(B)), trace=False)
    return np.stack([res.results[b]["out"] for b in range(B)], axis=0)


# revision 23
# speedup vs baseline: 1.0746x; 1.0110x over previous
"""Transformer encoder layer (nn_Encoder) on 8 TRN2 NeuronCores.

Strategy: data-parallel over batch — B=8, one batch element per core, weights
replicated, no collectives. Per core a single Bass/Tile kernel computes the
whole layer.

Precision plan: the attention path (QKV projections, softmax-weighted context,
Wo) runs in fp8e4m3 with DoubleRow matmuls (2 contraction tiles per
instruction, 0.5 cycles/row) — its error is attenuated ~25x by the residual
(|attn_out| ~ 0.04 |x|). Attention scores stay bf16. The FFN (whose output is
NOT small vs the residual) runs in bf16 at 1 cycle/row. All accumulation in
fp32 PSUM. fp8 operands are host-prescaled by 32 (weights are sigma~1/32 and
would hit the fp8 denormal floor); the combined 1024x scale on the attention
output is folded into the residual (x side is scaled 1024x instead) and
neutralized by LayerNorm's scale invariance.

Layout: attention runs in the "transposed domain" ([feature, tokens]) so every
weight matmul uses natural weight layouts; softmax over tokens-on-partitions is
handled by appending a ones-column to V (denominator lands in the ctx matmul's
extra output row, M=65). Per pair the 4 denominator rows are staged into one
[4, NS] tile, inverted with a single reciprocal, broadcast across partitions
with a tiny K=4 matmul against a selection matrix, and applied in one
full-width multiply per slice (deferred one pair to keep PE fed). Wo/FFN2
products land in the natural domain where both LayerNorms reduce along the
free dim; the normalize runs on ACT via per-partition scale/bias. b2 is folded
into be1 on the host (with b1 -= W1^T b2 compensating FFN1). W1/Wo are
prefetched into SBUF during the attention phase and W2 during FFN1, each read
from HBM exactly once. FFN2 runs si-major with per-si epilogues so only the
last tile's epilogue trails the final matmul.

Self-contained: hardcodes B=8, S=1024, D=1024, H=16, FF=2048, 8 cores.
"""
import math
import numpy as np
import ml_dtypes
from contextlib import ExitStack

import concourse.bass as bass
import concourse.tile as tile
from concourse import bacc, mybir
from concourse import bass_utils
from concourse.masks import make_identity

B = 8
S = 1024
D = 1024
H = 16
FF = 2048
P = 128
HD = 64
EPS = 1e-5
f32 = mybir.dt.float32
f32r = mybir.dt.float32r
bf16 = mybir.dt.bfloat16
f8 = mybir.dt.float8e4
np_bf16 = ml_dtypes.bfloat16
np_f8 = ml_dtypes.float8_e4m3
DR = mybir.MatmulPerfMode.DoubleRow
AF = mybir.ActivationFunctionType
ALU = mybir.AluOpType

NP_ = H // 2          # head pairs
ST = S // P           # token tiles
DT = D // P
D2 = DT // 2          # contraction pair-tiles over D
T2 = ST // 2          # token pair-tiles
FT = FF // P
NS = 512              # token slice width (matmul free dim)
SL = S // NS
ND = 512              # feature slice width
DL = D // ND

WSC = 32.0            # fp8 weight prescale (weights are sigma ~ 1/32)
RSC = WSC * WSC       # resulting scale on the attention output


def build_encoder(num_devices=8):
    # q,k arrive prescaled by WSC each: scores carry WSC^2; exp folds it into
    # its input scale. The extra -2ln2 bias keeps exp outputs < 240/4 (fp8e4m3
    # max is 240); softmax cancels any constant factor on e.
    e_scale = 1.0 / (math.sqrt(HD) * RSC)
    e_bias = -2.0 * math.log(2.0)
    nc = bacc.Bacc("TRN2", target_bir_lowering=False, debug=False,
                   enable_asserts=True, num_devices=num_devices)

    dram = lambda n, sh, dt: nc.dram_tensor(n, sh, dt, kind="ExternalInput").ap()
    xT_d = dram("xT", [D2, P, 2, S], f8)
    vones_d = dram("vones", [P, H], f8)
    sel_d = dram("sel", [SL, 4, P], f32r)
    x_d = dram("x", [S, D], f32)
    wq_d = dram("Wq", [NP_, P, D2, 2, P], f8)
    wk_d = dram("Wk", [NP_, P, D2, 2, P], f8)
    wv_d = dram("Wv", [D2, P, 2, D], f8)
    wo_d = dram("Wo", [D2, P, 2, D], f8)
    w1_d = dram("W1", [FT, P, DT, P], bf16)
    w2_d = dram("W2", [FF, D], bf16)
    bqc_d = dram("bqc", [P, NP_], f32)
    bkc_d = dram("bkc", [P, NP_], f32)
    b1c_d = dram("b1c", [P, FT], f32)
    bv_d = dram("bv", [D], f32)
    bo_d = dram("bo", [D], f32)
    g1_d = dram("g1", [D], f32)
    be1_d = dram("be1", [D], f32)
    g2_d = dram("g2", [D], f32)
    be2_d = dram("be2", [D], f32)
    out_d = nc.dram_tensor("out", [S, D], f32, kind="ExternalOutput").ap()

    with tile.TileContext(nc) as tc, ExitStack() as octx:
        const = octx.enter_context(tc.tile_pool(name="const", bufs=1))
        identity = const.tile([P, P], bf16, name="identity")
        make_identity(nc, identity)
        ebias = const.tile([P, 1], f32, name="ebias")
        nc.gpsimd.memset(ebias[:], e_bias)

        # long-lived weight pool (prefetched during attention) + h tiles
        pW = octx.enter_context(tc.tile_pool(name="pW", bufs=1))
        pH = octx.enter_context(tc.tile_pool(name="pH", bufs=1))

        def bcast_row(pool, name, src_row, width, tag=None):
            r = pool.tile([1, width], f32, name=f"{name}_r", tag="bcr", bufs=1)
            nc.sync.dma_start(r[:], src_row[None, :])
            b = pool.tile([P, width], f32, name=f"{name}_b",
                          tag=(tag or f"{name}_b"))
            nc.gpsimd.partition_broadcast(b[:], r[:])
            return b

        jview = lambda ap, w: ap.rearrange("p (j c) -> p j c", j=2)

        # ctxT pool (attention -> Wo), fp8 dv-pair tiles [P, 2, S]
        pCtx_cm = tc.tile_pool(name="pCtx", bufs=1)
        pCtx = pCtx_cm.__enter__()

        # ---------------- attention scope ----------------
        with tc.tile_pool(name="pA", bufs=1) as pA, \
             tc.tile_pool(name="psA", bufs=1, space="PSUM") as psA:

            # pair-0 Q/K weights first so QK(0) matmuls start ASAP
            wq0 = pA.tile([P, D2 * 2 * P], f8, name="wq0", tag="wq", bufs=2)
            nc.sync.dma_start(
                wq0[:].rearrange("p (d j m) -> p d j m", j=2, m=P), wq_d[0])
            wk0 = pA.tile([P, D2 * 2 * P], f8, name="wk0", tag="wk", bufs=2)
            nc.sync.dma_start(
                wk0[:].rearrange("p (d j m) -> p d j m", j=2, m=P), wk_d[0])

            # x^T fp8 pair tiles [P, 2, S]
            xt2 = []
            for d2 in range(D2):
                t = pA.tile([P, 2 * S], f8, name=f"xt{d2}", tag="xt", bufs=D2)
                nc.sync.dma_start(jview(t, S), xT_d[d2])
                xt2.append(t)

            # ---- V projection weights ----
            pExp_cm = tc.tile_pool(name="pExp", bufs=1)
            pExp = pExp_cm.__enter__()
            pV_cm = tc.tile_pool(name="pV", bufs=1)
            pV = pV_cm.__enter__()
            wv2 = []
            for d2 in range(D2):
                t = pV.tile([P, 2 * D], f8, name=f"wv{d2}", tag="wv", bufs=D2)
                nc.sync.dma_start(jview(t, D), wv_d[d2])
                wv2.append(t)

            # V65 token-pair tiles: [128, 2, H*65], ones at [:, :, 65h+64]
            v65 = []
            for t2 in range(T2):
                v = pA.tile([P, 2 * H * 65], f8, name=f"v65_{t2}", tag="v65",
                            bufs=T2)
                nc.gpsimd.memset(
                    jview(v, H * 65).rearrange(
                        "p j (h c) -> p j h c", c=65)[:, :, :, 64:65], 1.0)
                v65.append(v)

            # small consts (needed only after the first QK chains land)
            bqc = const.tile([P, NP_], f32, name="bqc")
            nc.sync.dma_start(bqc[:], bqc_d)
            bkc = const.tile([P, NP_], f32, name="bkc")
            nc.sync.dma_start(bkc[:], bkc_d)
            b1c = const.tile([P, FT], f32, name="b1c")
            nc.sync.dma_start(b1c[:], b1c_d)
            selt = const.tile([4, SL * P], f32r, name="selt")
            for sl in range(SL):
                nc.sync.dma_start(selt[:, sl * P:(sl + 1) * P], sel_d[sl])
            bv_b = bcast_row(pA, "bv", bv_d, D)

            # Wo prefetch (fp8 dv-pair tiles [P, 2, D])
            wo2 = []
            for q2 in range(D2):
                t = pW.tile([P, 2 * D], f8, name=f"wo{q2}", tag="wo", bufs=D2)
                nc.sync.dma_start(jview(t, D), wo_d[q2])
                wo2.append(t)

            # W1/W2 resident tiles; DMAs emitted later in the pair loop
            w1 = [pW.tile([P, DT * P], bf16, name=f"w1_{f}", tag="w1", bufs=FT)
                  for f in range(FT)]
            w2 = [pW.tile([P, D], bf16, name=f"w2_{f}", tag="w2", bufs=FT)
                  for f in range(FT)]

            def emit_w1_loads():
                for f in range(FT):
                    nc.sync.dma_start(
                        w1[f][:].rearrange("p (dt q) -> p dt q", q=P),
                        w1_d[f])

            def emit_w2_loads():
                for f in range(FT):
                    nc.sync.dma_start(w2[f][:], w2_d[f * P:(f + 1) * P, :])

            hpn = ND // HD
            v_state = {}

            def emit_v_chunk(hc):
                """Half-chunk hc of the V projection (chain = hc//2)."""
                chain = hc // 2
                part = hc % 2
                t, n = chain // DL, chain % DL
                if part == 0:
                    v_state[chain] = psA.tile(
                        [P, ND], f32, name=f"vps{t}_{n}", tag="vqk", bufs=2)
                ps = v_state[chain]
                for d2 in range(2 * part, 2 * part + 2):
                    nc.tensor.matmul(
                        ps[:], jview(xt2[d2], S)[:, :, t * P:(t + 1) * P],
                        jview(wv2[d2], D)[:, :, n * ND:(n + 1) * ND],
                        perf_mode=DR, start=(d2 == 0), stop=(d2 == D2 - 1))
                if part == 1:
                    dst = jview(v65[t // 2], H * 65).rearrange(
                        "p j (h c) -> p j h c", c=65)[
                        :, t % 2, n * hpn:(n + 1) * hpn, 0:64]
                    srcv = ps[:].rearrange("p (h k) -> p h k", k=HD)
                    bvs = bv_b[:, n * ND:(n + 1) * ND].rearrange(
                        "p (h k) -> p h k", k=HD)
                    nc.vector.tensor_add(dst, srcv, bvs)

            # ---- attention per head pair ----
            ctxT2 = [pCtx.tile([P, 2 * S], f8, name=f"ctxT{q2}", tag="ctxT",
                               bufs=D2) for q2 in range(D2)]

            def emit_normalize(p, ctxU, den4):
                """Deferred softmax-normalize of pair p's ctx."""
                den4r = pA.tile([4, NS], f32r, name=f"den4r_{p}", tag="den4r",
                                bufs=2)
                with nc.allow_low_precision("softmax denom recip in f32r"):
                    nc.vector.reciprocal(den4r[:], den4[:])
                for sl in range(SL):
                    rcb = psA.tile([P, NS], f32, name=f"rcb{p}_{sl}",
                                   tag="vqk", bufs=2)
                    nc.tensor.matmul(rcb[:], selt[:, sl * P:(sl + 1) * P],
                                     den4r[:], start=True, stop=True)
                    nc.vector.tensor_mul(
                        jview(ctxT2[p // 2], S)[
                            :, p % 2, sl * NS:(sl + 1) * NS],
                        ctxU[:, sl * NS:(sl + 1) * NS], rcb[:])

            def emit_qk_chain_part(p, chain, part, state):
                """Emit 2 of the 4 DoubleRow accumulation matmuls of QK chain
                (chain: 0..3 = Q-sl0, Q-sl1, K-sl0, K-sl1) for pair p."""
                wt, bc, dst = state["ops"][chain // 2]
                sl = chain % 2
                if part == 0:
                    state[chain] = psA.tile(
                        [P, NS], f32, name=f"qk{p}_{chain}", tag="vqk", bufs=2)
                ps = state[chain]
                wtv = wt[:].rearrange("p (d j m) -> p d j m", j=2, m=P)
                for d2 in range(2 * part, 2 * part + 2):
                    nc.tensor.matmul(
                        ps[:], wtv[:, d2],
                        jview(xt2[d2], S)[:, :, sl * NS:(sl + 1) * NS],
                        perf_mode=DR, start=(d2 == 0), stop=(d2 == D2 - 1))
                if part == 1:
                    nc.vector.tensor_scalar(
                        out=dst[:, sl * NS:(sl + 1) * NS], in0=ps[:],
                        scalar1=bc[:, p:p + 1], scalar2=None, op0=ALU.add)

            def make_qk_state(p):
                if p == 0:
                    wqt, wkt = wq0, wk0
                else:
                    wqt = pA.tile([P, D2 * 2 * P], f8, name=f"wq{p}",
                                  tag="wq", bufs=2)
                    nc.sync.dma_start(
                        wqt[:].rearrange("p (d j m) -> p d j m", j=2, m=P),
                        wq_d[p])
                    wkt = pA.tile([P, D2 * 2 * P], f8, name=f"wk{p}",
                                  tag="wk", bufs=2)
                    nc.sync.dma_start(
                        wkt[:].rearrange("p (d j m) -> p d j m", j=2, m=P),
                        wk_d[p])
                qt = pA.tile([P, S], f8, name=f"qt{p}", tag="qt", bufs=2)
                kt = pA.tile([P, S], f8, name=f"kt{p}", tag="kt", bufs=2)
                return {"ops": ((wqt, bqc, qt), (wkt, bkc, kt)),
                        "qt": qt, "kt": kt}

            LAG = 2
            qk_state = make_qk_state(0)
            for chain in range(4):
                for part in range(2):
                    emit_qk_chain_part(0, chain, part, qk_state)

            pending = None
            for p in range(NP_):
                if p == 2:
                    emit_w1_loads()
                if p == 5:
                    emit_w2_loads()
                qt, kt = qk_state["qt"], qk_state["kt"]
                next_state = make_qk_state(p + 1) if p + 1 < NP_ else None

                ctxU = pA.tile([P, S], f32, name=f"ctxU{p}", tag="ctxU",
                               bufs=2)
                den4 = pA.tile([4, NS], f32, name=f"den4_{p}", tag="den4",
                               bufs=2)

                def emit_scores(sl, t, expt):
                    ps = psA.tile([P, 2 * NS], f32, name=f"sc{t}_{sl}",
                                  tag="sc", bufs=2)
                    for h in range(2):
                        nc.tensor.matmul(
                            ps[:, h * NS:(h + 1) * NS],
                            kt[h * HD:(h + 1) * HD, t * P:(t + 1) * P],
                            qt[h * HD:(h + 1) * HD, sl * NS:(sl + 1) * NS],
                            start=True, stop=True,
                            tile_position=(h * HD, 0))
                    if t % 2 == 0:
                        expt[t // 2] = pExp.tile(
                            [P, 2 * 2 * NS], f8, name=f"e{t // 2}_{sl}",
                            tag="exp", bufs=3)
                    e2 = expt[t // 2]
                    nc.scalar.activation(jview(e2, 2 * NS)[:, t % 2, :],
                                         ps[:], AF.Exp, scale=e_scale,
                                         bias=ebias[:])

                def emit_ctx(sl, t2, cps, expt):
                    for h in range(2):
                        lhs = jview(v65[t2], H * 65)[
                            :, :, (2 * p + h) * 65:(2 * p + h) * 65 + 65]
                        nc.tensor.matmul(
                            cps[h][0:65, :], lhs,
                            jview(expt[t2], 2 * NS)[
                                :, :, h * NS:(h + 1) * NS],
                            perf_mode=DR,
                            start=(t2 == 0), stop=(t2 == T2 - 1))

                def emit_evict(sl, cps):
                    for h in range(2):
                        ps = cps[h]
                        stage = pA.tile([65, NS], f32, name=f"stg{h}{sl}",
                                        tag="rc", bufs=2)
                        nc.vector.tensor_copy(stage[64:65, :], ps[64:65, :])
                        nc.sync.dma_start(
                            den4[h * SL + sl:h * SL + sl + 1, :],
                            stage[64:65, :])
                        if h == 0:
                            nc.vector.tensor_copy(
                                ctxU[0:HD, sl * NS:(sl + 1) * NS],
                                ps[0:HD, :])
                        else:
                            tmp = pA.tile([HD, NS], f32, name=f"ctmp{sl}",
                                          tag="ctmp", bufs=2)
                            nc.vector.tensor_copy(tmp[:], ps[0:HD, :])
                            nc.sync.dma_start(
                                ctxU[HD:P, sl * NS:(sl + 1) * NS], tmp[:])

                expt0 = {}
                cps0 = [psA.tile([P, NS], f32, name=f"cps{h}_0", tag="ctx",
                                 bufs=2) for h in range(2)]
                expt1 = {}
                cps1 = [psA.tile([P, NS], f32, name=f"cps{h}_1", tag="ctx",
                                 bufs=2) for h in range(2)]
                if p == 0:
                    # A: scores(sl0) + the whole V projection interleaved
                    for t in range(ST):
                        emit_scores(0, t, expt0)
                        for hc in range(4 * t, 4 * t + 4):
                            emit_v_chunk(hc)
                    # B: scores(sl1) + lagged ctx(sl0) per token pair
                    for t in range(ST + LAG):
                        if t < ST:
                            emit_scores(1, t, expt1)
                        if t >= LAG and (t - LAG) % 2 == 1:
                            emit_ctx(0, (t - LAG) // 2, cps0, expt0)
                    emit_evict(0, cps0)
                    # C: ctx(sl1) + QK(1) chunks
                    for t in range(ST):
                        if t % 2 == 1:
                            emit_ctx(1, t // 2, cps1, expt1)
                        if next_state is not None:
                            emit_qk_chain_part(p + 1, t // 2, t % 2,
                                               next_state)
                    emit_evict(1, cps1)
                    pV_cm.__exit__(None, None, None)
                else:
                    # A: scores(sl0) + QK(p+1) chunks 0-3 + lagged ctx(sl0)
                    for t in range(ST + LAG):
                        if t < ST:
                            emit_scores(0, t, expt0)
                            if next_state is not None and t < 4:
                                emit_qk_chain_part(p + 1, t // 2, t % 2,
                                                   next_state)
                        if t >= LAG and (t - LAG) % 2 == 1:
                            emit_ctx(0, (t - LAG) // 2, cps0, expt0)
                    emit_evict(0, cps0)
                    if pending is not None:
                        emit_normalize(*pending)
                    # B: scores(sl1) + QK(p+1) chunks 4-7 + lagged ctx(sl1)
                    for t in range(ST + LAG):
                        if t < ST:
                            emit_scores(1, t, expt1)
                            if next_state is not None and t < 4:
                                emit_qk_chain_part(p + 1, (t + 4) // 2,
                                                   t % 2, next_state)
                        if t >= LAG and (t - LAG) % 2 == 1:
                            emit_ctx(1, (t - LAG) // 2, cps1, expt1)
                    emit_evict(1, cps1)
                pending = (p, ctxU, den4)
                qk_state = next_state
            emit_normalize(*pending)
            pExp_cm.__exit__(None, None, None)

        # -------- Wo + LN1 + FFN merged scope (PE never starves) --------
        # hn = LN1(RSC*(x + bo) + ctxT2@wo2) * g1 + (be1 + b2); the RSC scale
        # on both residual and product cancels in LayerNorm. FFN1 half-0
        # chains are interleaved under the LN1/transpose epilogues of si 4-7
        # (the fp8 Wo matmuls alone cannot keep the PE fed).
        h_nat = []
        ht = [pH.tile([P, S], bf16, name=f"ht{d}", tag="ht", bufs=DT)
              for d in range(DT)]
        with tc.tile_pool(name="pWo", bufs=1) as pWo:
            psW_cm = tc.tile_pool(name="psW", bufs=1, space="PSUM")
            psW = psW_cm.__enter__()
            psU_cm = tc.tile_pool(name="psU", bufs=1, space="PSUM")
            psU = psU_cm.__enter__()
            bo_b = bcast_row(pWo, "bo", bo_d, D, tag="bc0")
            g1_b = bcast_row(pWo, "g1", g1_d, D, tag="bc1")
            be1_b = bcast_row(pWo, "be1", be1_d, D, tag="bc2")

            # residual tiles: RSC*(x + bo)
            xns = []
            for si in range(ST):
                xn = pWo.tile([P, D], f32, name=f"xn{si}", tag="xn", bufs=3)
                nc.sync.dma_start(xn[:], x_d[si * P:(si + 1) * P, :])
                nc.vector.tensor_add(xn[:], xn[:], bo_b[:])
                nc.vector.tensor_scalar_mul(xn[:], xn[:], RSC)
                xns.append(xn)

            ut = [pWo.tile([P, S], bf16, name=f"ut{f}", tag="ut", bufs=FT)
                  for f in range(FT)]

            def emit_ffn1(f, hf):
                ps = psU.tile([P, NS], f32, name=f"u{f}_{hf}", tag="u",
                              bufs=4)
                for d in range(DT):
                    nc.tensor.matmul(
                        ps[:], w1[f][:, d * P:(d + 1) * P],
                        ht[d][:, hf * NS:(hf + 1) * NS],
                        start=(d == 0), stop=(d == DT - 1))
                nc.scalar.activation(ut[f][:, hf * NS:(hf + 1) * NS],
                                     ps[:], AF.Relu, bias=b1c[:, f:f + 1])

            def ln_stats(pool, si, v, pfx):
                st = pool.tile([P, 8], f32, name=f"{pfx}st{si}", tag="st",
                               bufs=4)
                s1 = st[:, 2:3]
                s2 = st[:, 3:4]; mu = st[:, 4:5]; var = st[:, 5:6]
                rstd = st[:, 6:7]; nm = st[:, 7:8]
                scr = pool.tile([P, D], f32, name=f"{pfx}scr{si}", tag="scr",
                                bufs=2)
                nc.scalar.activation(scr[:], v[:], AF.Copy, accum_out=s1)
                nc.scalar.activation(scr[:], v[:], AF.Square, accum_out=s2)
                nc.vector.tensor_scalar_mul(mu, s1, 1.0 / D)
                nc.vector.tensor_scalar_mul(var, s2, 1.0 / D)
                nc.vector.tensor_mul(nm, mu, mu)
                nc.vector.tensor_sub(var, var, nm)
                nc.vector.tensor_scalar_add(var, var, EPS)
                nc.scalar.sqrt(var, var)
                nc.vector.reciprocal(rstd, var)
                nc.vector.tensor_mul(nm, mu, rstd)
                nc.vector.tensor_scalar_mul(nm, nm, -1.0)
                nc.scalar.activation(scr[:], v[:], AF.Identity,
                                     bias=nm, scale=rstd)
                return scr

            def emit_wo_si(si):
                pss = [psW.tile([P, ND], f32, name=f"c{si}_{n}", tag="c",
                                bufs=2) for n in range(DL)]
                for q2 in range(D2):
                    for n in range(DL):
                        nc.tensor.matmul(
                            pss[n][:],
                            jview(ctxT2[q2], S)[:, :, si * P:(si + 1) * P],
                            jview(wo2[q2], D)[:, :, n * ND:(n + 1) * ND],
                            perf_mode=DR,
                            start=(q2 == 0), stop=(q2 == D2 - 1))
                v = pWo.tile([P, D], f32, name=f"v{si}", tag="v", bufs=3)
                for n in range(DL):
                    nc.vector.tensor_add(
                        v[:, n * ND:(n + 1) * ND], pss[n][:],
                        xns[si][:, n * ND:(n + 1) * ND])
                scr = ln_stats(pWo, si, v, "ln1")
                hn = pH.tile([P, D], bf16, name=f"hn{si}", tag="hn", bufs=ST)
                nc.vector.tensor_mul(scr[:], scr[:], g1_b[:])
                nc.vector.tensor_add(hn[:], scr[:], be1_b[:])
                h_nat.append(hn)
                # h^T transposes for this si; copybacks alternate DVE/ACT
                for dd in range(DT):
                    ps = psW.tile([P, P], bf16, name=f"tp{si}_{dd}", tag="tp",
                                  bufs=2)
                    nc.tensor.transpose(
                        ps[:], hn[:, dd * P:(dd + 1) * P], identity[:])
                    dst = ht[dd][:, si * P:(si + 1) * P]
                    if dd % 2 == 0:
                        nc.vector.tensor_copy(dst, ps[:])
                    else:
                        nc.scalar.copy(dst, ps[:])

            for si in range(ST):
                emit_wo_si(si)
                if 3 <= si <= 6:
                    for f in range(4 * (si - 3), 4 * (si - 3) + 4):
                        emit_ffn1(f, 0)

            g2_b = bcast_row(pWo, "g2", g2_d, D, tag="bc0")
            be2_b = bcast_row(pWo, "be2", be2_d, D, tag="bc1")
            for f in range(FT):
                emit_ffn1(f, 1)
            psU_cm.__exit__(None, None, None)
            psW_cm.__exit__(None, None, None)

            # ---- FFN2 si-major with per-si LN2 epilogue ----
            with tc.tile_pool(name="psY", bufs=1, space="PSUM") as psY:
                for si in range(ST):
                    pss = [psY.tile([P, ND], f32, name=f"y{si}_{n}", tag="y",
                                    bufs=6) for n in range(DL)]
                    for f in range(FT):
                        for n in range(DL):
                            nc.tensor.matmul(
                                pss[n][:],
                                ut[f][:, si * P:(si + 1) * P],
                                w2[f][:, n * ND:(n + 1) * ND],
                                start=(f == 0), stop=(f == FT - 1))
                    v = pWo.tile([P, D], f32, name=f"v2_{si}", tag="v",
                                 bufs=3)
                    for n in range(DL):
                        nc.vector.tensor_add(
                            v[:, n * ND:(n + 1) * ND], pss[n][:],
                            h_nat[si][:, n * ND:(n + 1) * ND])
                    scr = ln_stats(pWo, si, v, "ln2")
                    o = pWo.tile([P, D], f32, name=f"o{si}", tag="o", bufs=3)
                    nc.vector.tensor_mul(scr[:], scr[:], g2_b[:])
                    nc.vector.tensor_add(o[:], scr[:], be2_b[:])
                    nc.sync.dma_start(out_d[si * P:(si + 1) * P, :], o[:])
        pCtx_cm.__exit__(None, None, None)

    nc.compile()
    return nc


def pack_core_inputs(x_b, shared):
    """Per-core input map: batch element x_b + shared (prepacked) weights."""
    m = dict(shared)
    x_b = np.asarray(x_b, dtype=np.float32)
    m["x"] = np.ascontiguousarray(x_b)
    # x^T in fp8, dv-pair interleaved: [D2, P, 2, S]
    xT = np.ascontiguousarray(x_b.T)
    m["xT"] = np.ascontiguousarray(
        xT.reshape(D2, 2, P, S).transpose(0, 2, 1, 3).astype(np_f8))
    return m


def pack_shared(Wq, bq, Wk, bk, Wv, bv, Wo, bo, ln1_g, ln1_b, W1, b1, W2, b2,
                ln2_g, ln2_b):
    """Host-side layout packing of the replicated weights. fp8 weights are
    prescaled by WSC=32 (their sigma is ~1/32); be1 absorbs b2 exactly with
    b1 -= W1^T b2 compensating FFN1."""
    f = np.float32
    Wq = np.asarray(Wq, dtype=f); Wk = np.asarray(Wk, dtype=f)
    Wv = np.asarray(Wv, dtype=f); Wo = np.asarray(Wo, dtype=f)
    W1 = np.asarray(W1, dtype=f); W2 = np.asarray(W2, dtype=f)
    b1 = np.asarray(b1, dtype=f); b2 = np.asarray(b2, dtype=f)
    ln1_b = np.asarray(ln1_b, dtype=f)
    # [pair, p, d2, j, m]: DoubleRow dv-pair layout, contiguous per-pair DMA
    pack_qk = lambda W: np.ascontiguousarray(
        (WSC * W).reshape(D, H * HD).reshape(D2, 2, P, NP_, P).transpose(
            3, 2, 0, 1, 4).astype(np_f8))
    pack_dv = lambda W: np.ascontiguousarray(
        (WSC * W).reshape(D2, 2, P, D).transpose(0, 2, 1, 3).astype(np_f8))
    sel = np.zeros((SL, 4, P), dtype=f)
    for sl in range(SL):
        for m in range(P):
            sel[sl, (m // HD) * SL + sl, m] = 1.0
    be1_f = ln1_b + b2
    b1_f = (b1.astype(np.float64) -
            W1.astype(np.float64).T @ b2.astype(np.float64)).astype(f)
    return {
        "vones": np.ones((P, H), dtype=np_f8),
        "sel": sel,
        "Wq": pack_qk(Wq), "Wk": pack_qk(Wk),
        "Wv": pack_dv(Wv.reshape(D, D)),
        "Wo": pack_dv(Wo),
        "W1": np.ascontiguousarray(
            W1.reshape(DT, P, FT, P).transpose(2, 1, 0, 3)).astype(np_bf16),
        "W2": np.ascontiguousarray(W2).astype(np_bf16),
        "bqc": np.ascontiguousarray(WSC * np.asarray(bq, f).reshape(NP_, P).T),
        "bkc": np.ascontiguousarray(WSC * np.asarray(bk, f).reshape(NP_, P).T),
        "b1c": np.ascontiguousarray(b1_f.reshape(FT, P).T),
        "bv": np.ascontiguousarray(WSC * np.asarray(bv, f).reshape(D)),
        "bo": np.ascontiguousarray(bo, dtype=f),
        "g1": np.ascontiguousarray(ln1_g, dtype=f),
        "be1": np.ascontiguousarray(be1_f),
        "g2": np.ascontiguousarray(ln2_g, dtype=f),
        "be2": np.ascontiguousarray(ln2_b, dtype=f),
    }


_NC_CACHE = {}


def get_nc():
    if "nc" not in _NC_CACHE:
        _NC_CACHE["nc"] = build_encoder(num_devices=8)
    return _NC_CACHE["nc"]


def kernel(x, Wq, bq, Wk, bk, Wv, bv, Wo, bo, ln1_g, ln1_b, W1, b1, W2, b2,
           ln2_g, ln2_b):
    x = np.asarray(x)
    assert x.shape == (B, S, D)
    shared = pack_shared(Wq, bq, Wk, bk, Wv, bv, Wo, bo, ln1_g, ln1_b,
                         W1, b1, W2, b2, ln2_g, ln2_b)
    in_maps = [pack_core_inputs(x[b], shared) for b in range(B)]
    nc = get_nc()
    res = bass_utils.run_bass_kernel_spmd(
        nc, in_maps, core_ids=list(range(B)), trace=False)
    return np.stack([res.results[b]["out"] for b in range(B)], axis=0)


# revision 25
# speedup vs baseline: 1.1527x; 1.0727x over previous
"""Transformer encoder layer (nn_Encoder) on 8 TRN2 NeuronCores.

Strategy: data-parallel over batch — B=8, one batch element per core, weights
replicated, no collectives. Per core a single Bass/Tile kernel computes the
whole layer.

Precision plan: the attention path (QKV projections, softmax-weighted context,
Wo) runs in fp8e4m3 with DoubleRow matmuls (2 contraction tiles per
instruction, 0.5 cycles/row) — its error is attenuated ~25x by the residual
(|attn_out| ~ 0.04 |x|). Attention scores stay bf16. The FFN (whose output is
NOT small vs the residual) runs in bf16 at 1 cycle/row. All accumulation in
fp32 PSUM. fp8 operands are host-prescaled by 32 (weights are sigma~1/32 and
would hit the fp8 denormal floor); the combined 1024x scale on the attention
output is folded into the residual (x side is scaled 1024x instead) and
neutralized by LayerNorm's scale invariance.

Layout: attention runs in the "transposed domain" ([feature, tokens]) so every
weight matmul uses natural weight layouts; softmax over tokens-on-partitions is
handled by appending a ones-column to V (denominator lands in the ctx matmul's
extra output row, M=65). Per pair the 4 denominator rows are staged into one
[4, NS] tile, inverted with a single reciprocal, broadcast across partitions
with a tiny K=4 matmul against a selection matrix, and applied in one
full-width multiply per slice (deferred one pair to keep PE fed). Wo/FFN2
products land in the natural domain where both LayerNorms reduce along the
free dim; the normalize runs on ACT via per-partition scale/bias. b2 is folded
into be1 on the host (with b1 -= W1^T b2 compensating FFN1). W1/Wo are
prefetched into SBUF during the attention phase and W2 during FFN1, each read
from HBM exactly once. FFN2 runs si-major with per-si epilogues so only the
last tile's epilogue trails the final matmul.

Self-contained: hardcodes B=8, S=1024, D=1024, H=16, FF=2048, 8 cores.
"""
import math
import numpy as np
import ml_dtypes
from contextlib import ExitStack

import concourse.bass as bass
import concourse.tile as tile
from concourse import bacc, mybir
from concourse import bass_utils
from concourse.masks import make_identity

B = 8
S = 1024
D = 1024
H = 16
FF = 2048
P = 128
HD = 64
EPS = 1e-5
f32 = mybir.dt.float32
f32r = mybir.dt.float32r
bf16 = mybir.dt.bfloat16
f8 = mybir.dt.float8e4
np_bf16 = ml_dtypes.bfloat16
np_f8 = ml_dtypes.float8_e4m3
DR = mybir.MatmulPerfMode.DoubleRow
AF = mybir.ActivationFunctionType
ALU = mybir.AluOpType

NP_ = H // 2          # head pairs
ST = S // P           # token tiles
DT = D // P
D2 = DT // 2          # contraction pair-tiles over D
T2 = ST // 2          # token pair-tiles
FT = FF // P
NS = 512              # token slice width (matmul free dim)
SL = S // NS
ND = 512              # feature slice width
DL = D // ND

WSC = 32.0            # fp8 weight prescale (weights are sigma ~ 1/32)
RSC = WSC * WSC       # resulting scale on the attention output


def build_encoder(num_devices=8):
    # q,k arrive prescaled by WSC each: scores carry WSC^2; exp folds it into
    # its input scale. The extra -2ln2 bias keeps exp outputs < 240/4 (fp8e4m3
    # max is 240); softmax cancels any constant factor on e.
    e_scale = 1.0 / (math.sqrt(HD) * RSC)
    e_bias = -2.0 * math.log(2.0)
    nc = bacc.Bacc("TRN2", target_bir_lowering=False, debug=False,
                   enable_asserts=True, num_devices=num_devices)

    dram = lambda n, sh, dt: nc.dram_tensor(n, sh, dt, kind="ExternalInput").ap()
    xT_d = dram("xT", [D2, P, 2, S], f8)
    vones_d = dram("vones", [P, H], f8)
    sel_d = dram("sel", [SL, 4, P], bf16)
    x_d = dram("x", [S, D], f32)
    wq_d = dram("Wq", [NP_, P, D2, 2, P], f8)
    wk_d = dram("Wk", [NP_, P, D2, 2, P], f8)
    wv_d = dram("Wv", [D2, P, 2, D], f8)
    wo_d = dram("Wo", [D2, P, 2, D], f8)
    w1_d = dram("W1", [FT, P, DT, P], bf16)
    w2_d = dram("W2", [FF, D], bf16)
    bqc_d = dram("bqc", [P, NP_], f32)
    bkc_d = dram("bkc", [P, NP_], f32)
    b1c_d = dram("b1c", [P, FT], f32)
    bv_d = dram("bv", [D], f32)
    bo_d = dram("bo", [D], f32)
    g1_d = dram("g1", [D], f32)
    be1_d = dram("be1", [D], f32)
    g2_d = dram("g2", [D], f32)
    be2_d = dram("be2", [D], f32)
    out_d = nc.dram_tensor("out", [S, D], f32, kind="ExternalOutput").ap()

    with tile.TileContext(nc) as tc, ExitStack() as octx:
        const = octx.enter_context(tc.tile_pool(name="const", bufs=1))
        identity = const.tile([P, P], bf16, name="identity")
        make_identity(nc, identity)
        ebias = const.tile([P, 1], f32, name="ebias")
        nc.gpsimd.memset(ebias[:], e_bias)

        # long-lived weight pool (prefetched during attention) + h tiles
        pW = octx.enter_context(tc.tile_pool(name="pW", bufs=1))
        pH = octx.enter_context(tc.tile_pool(name="pH", bufs=1))

        def bcast_row(pool, name, src_row, width, tag=None):
            r = pool.tile([1, width], f32, name=f"{name}_r", tag="bcr", bufs=1)
            nc.sync.dma_start(r[:], src_row[None, :])
            b = pool.tile([P, width], f32, name=f"{name}_b",
                          tag=(tag or f"{name}_b"))
            nc.gpsimd.partition_broadcast(b[:], r[:])
            return b

        jview = lambda ap, w: ap.rearrange("p (j c) -> p j c", j=2)

        # ctxT pool (attention -> Wo), fp8 dv-pair tiles [P, 2, S]
        pCtx_cm = tc.tile_pool(name="pCtx", bufs=1)
        pCtx = pCtx_cm.__enter__()

        # ---------------- attention scope ----------------
        with tc.tile_pool(name="pA", bufs=1) as pA, \
             tc.tile_pool(name="psA", bufs=1, space="PSUM") as psA:

            # pair-0 Q/K weights first so QK(0) matmuls start ASAP
            wq0 = pA.tile([P, D2 * 2 * P], f8, name="wq0", tag="wq", bufs=2)
            nc.sync.dma_start(
                wq0[:].rearrange("p (d j m) -> p d j m", j=2, m=P), wq_d[0])
            wk0 = pA.tile([P, D2 * 2 * P], f8, name="wk0", tag="wk", bufs=2)
            nc.sync.dma_start(
                wk0[:].rearrange("p (d j m) -> p d j m", j=2, m=P), wk_d[0])

            # x^T fp8 pair tiles [P, 2, S]
            xt2 = []
            for d2 in range(D2):
                t = pA.tile([P, 2 * S], f8, name=f"xt{d2}", tag="xt", bufs=D2)
                nc.sync.dma_start(jview(t, S), xT_d[d2])
                xt2.append(t)

            # ---- V projection weights ----
            pExp_cm = tc.tile_pool(name="pExp", bufs=1)
            pExp = pExp_cm.__enter__()
            pV_cm = tc.tile_pool(name="pV", bufs=1)
            pV = pV_cm.__enter__()
            wv2 = []
            for d2 in range(D2):
                t = pV.tile([P, 2 * D], f8, name=f"wv{d2}", tag="wv", bufs=D2)
                nc.sync.dma_start(jview(t, D), wv_d[d2])
                wv2.append(t)

            # V65 token-pair tiles: [128, 2, H*65], ones at [:, :, 65h+64]
            v65 = []
            for t2 in range(T2):
                v = pA.tile([P, 2 * H * 65], f8, name=f"v65_{t2}", tag="v65",
                            bufs=T2)
                nc.gpsimd.memset(
                    jview(v, H * 65).rearrange(
                        "p j (h c) -> p j h c", c=65)[:, :, :, 64:65], 1.0)
                v65.append(v)

            # small consts (needed only after the first QK chains land)
            bqc = const.tile([P, NP_], f32, name="bqc")
            nc.sync.dma_start(bqc[:], bqc_d)
            bkc = const.tile([P, NP_], f32, name="bkc")
            nc.sync.dma_start(bkc[:], bkc_d)
            b1c = const.tile([P, FT], f32, name="b1c")
            nc.sync.dma_start(b1c[:], b1c_d)
            selt = const.tile([4, SL * P], bf16, name="selt")
            for sl in range(SL):
                nc.sync.dma_start(selt[:, sl * P:(sl + 1) * P], sel_d[sl])
            bv_b = bcast_row(pA, "bv", bv_d, D)

            # Wo prefetch (fp8 dv-pair tiles [P, 2, D])
            wo2 = []
            for q2 in range(D2):
                t = pW.tile([P, 2 * D], f8, name=f"wo{q2}", tag="wo", bufs=D2)
                nc.sync.dma_start(jview(t, D), wo_d[q2])
                wo2.append(t)

            # W1/W2 resident tiles; DMAs emitted later in the pair loop
            w1 = [pW.tile([P, DT * P], bf16, name=f"w1_{f}", tag="w1", bufs=FT)
                  for f in range(FT)]
            w2 = [pW.tile([P, D], bf16, name=f"w2_{f}", tag="w2", bufs=FT)
                  for f in range(FT)]

            def emit_w1_loads():
                for f in range(FT):
                    nc.sync.dma_start(
                        w1[f][:].rearrange("p (dt q) -> p dt q", q=P),
                        w1_d[f])

            def emit_w2_loads():
                for f in range(FT):
                    nc.sync.dma_start(w2[f][:], w2_d[f * P:(f + 1) * P, :])

            hpn = ND // HD
            v_state = {}

            def emit_v_chunk(hc):
                """Half-chunk hc of the V projection (chain = hc//2)."""
                chain = hc // 2
                part = hc % 2
                t, n = chain // DL, chain % DL
                if part == 0:
                    v_state[chain] = psA.tile(
                        [P, ND], f32, name=f"vps{t}_{n}", tag="vqk", bufs=2)
                ps = v_state[chain]
                for d2 in range(2 * part, 2 * part + 2):
                    nc.tensor.matmul(
                        ps[:], jview(xt2[d2], S)[:, :, t * P:(t + 1) * P],
                        jview(wv2[d2], D)[:, :, n * ND:(n + 1) * ND],
                        perf_mode=DR, start=(d2 == 0), stop=(d2 == D2 - 1))
                if part == 1:
                    dst = jview(v65[t // 2], H * 65).rearrange(
                        "p j (h c) -> p j h c", c=65)[
                        :, t % 2, n * hpn:(n + 1) * hpn, 0:64]
                    srcv = ps[:].rearrange("p (h k) -> p h k", k=HD)
                    bvs = bv_b[:, n * ND:(n + 1) * ND].rearrange(
                        "p (h k) -> p h k", k=HD)
                    nc.vector.tensor_add(dst, srcv, bvs)

            # ---- attention per head pair ----
            ctxT2 = [pCtx.tile([P, 2 * S], f8, name=f"ctxT{q2}", tag="ctxT",
                               bufs=D2) for q2 in range(D2)]

            def emit_normalize(p, ctxU, den128):
                """Deferred softmax-normalize of pair p's ctx. The
                denominators live as [128, 16] so the reciprocal runs wide;
                a reshape DMA lays them back out as the [4, NS] rcb moving
                operand."""
                denr = pA.tile([P, 16], bf16, name=f"denr_{p}", tag="denr",
                               bufs=2)
                with nc.allow_low_precision("softmax denom recip in bf16"):
                    nc.vector.reciprocal(denr[:], den128[:])
                den4r = pA.tile([4, NS], bf16, name=f"den4r_{p}", tag="den4r",
                                bufs=2)
                for r in range(4):
                    nc.sync.dma_start(den4r[r:r + 1, :], denr[:, 4 * r:4 * r + 4])
                for sl in range(SL):
                    rcb = psA.tile([P, NS], f32, name=f"rcb{p}_{sl}",
                                   tag="vqk", bufs=2)
                    nc.tensor.matmul(rcb[:], selt[:, sl * P:(sl + 1) * P],
                                     den4r[:], start=True, stop=True)
                    nc.vector.tensor_mul(
                        jview(ctxT2[p // 2], S)[
                            :, p % 2, sl * NS:(sl + 1) * NS],
                        ctxU[:, sl * NS:(sl + 1) * NS], rcb[:])

            def emit_qk_chain_part(p, chain, part, state):
                """Emit 2 of the 4 DoubleRow accumulation matmuls of QK chain
                (chain: 0..3 = Q-sl0, Q-sl1, K-sl0, K-sl1) for pair p."""
                wt, bc, dst = state["ops"][chain // 2]
                sl = chain % 2
                if part == 0:
                    state[chain] = psA.tile(
                        [P, NS], f32, name=f"qk{p}_{chain}", tag="vqk", bufs=2)
                ps = state[chain]
                wtv = wt[:].rearrange("p (d j m) -> p d j m", j=2, m=P)
                for d2 in range(2 * part, 2 * part + 2):
                    nc.tensor.matmul(
                        ps[:], wtv[:, d2],
                        jview(xt2[d2], S)[:, :, sl * NS:(sl + 1) * NS],
                        perf_mode=DR, start=(d2 == 0), stop=(d2 == D2 - 1))
                if part == 1:
                    nc.vector.tensor_scalar(
                        out=dst[:, sl * NS:(sl + 1) * NS], in0=ps[:],
                        scalar1=bc[:, p:p + 1], scalar2=None, op0=ALU.add)

            def make_qk_state(p):
                if p == 0:
                    wqt, wkt = wq0, wk0
                else:
                    wqt = pA.tile([P, D2 * 2 * P], f8, name=f"wq{p}",
                                  tag="wq", bufs=2)
                    nc.sync.dma_start(
                        wqt[:].rearrange("p (d j m) -> p d j m", j=2, m=P),
                        wq_d[p])
                    wkt = pA.tile([P, D2 * 2 * P], f8, name=f"wk{p}",
                                  tag="wk", bufs=2)
                    nc.sync.dma_start(
                        wkt[:].rearrange("p (d j m) -> p d j m", j=2, m=P),
                        wk_d[p])
                qt = pA.tile([P, S], f8, name=f"qt{p}", tag="qt", bufs=2)
                kt = pA.tile([P, S], f8, name=f"kt{p}", tag="kt", bufs=2)
                return {"ops": ((wqt, bqc, qt), (wkt, bkc, kt)),
                        "qt": qt, "kt": kt}

            LAG = 2
            qk_state = make_qk_state(0)
            for chain in range(4):
                for part in range(2):
                    emit_qk_chain_part(0, chain, part, qk_state)

            pending = None
            for p in range(NP_):
                if p == 2:
                    emit_w1_loads()
                if p == 5:
                    emit_w2_loads()
                qt, kt = qk_state["qt"], qk_state["kt"]
                next_state = make_qk_state(p + 1) if p + 1 < NP_ else None

                ctxU = pA.tile([P, S], f32, name=f"ctxU{p}", tag="ctxU",
                               bufs=2)
                den128 = pA.tile([P, 16], f32, name=f"den128_{p}", tag="den",
                                 bufs=2)

                def emit_scores(sl, t, expt):
                    ps = psA.tile([P, 2 * NS], f32, name=f"sc{t}_{sl}",
                                  tag="sc", bufs=2)
                    for h in range(2):
                        nc.tensor.matmul(
                            ps[:, h * NS:(h + 1) * NS],
                            kt[h * HD:(h + 1) * HD, t * P:(t + 1) * P],
                            qt[h * HD:(h + 1) * HD, sl * NS:(sl + 1) * NS],
                            start=True, stop=True,
                            tile_position=(h * HD, 0))
                    if t % 2 == 0:
                        expt[t // 2] = pExp.tile(
                            [P, 2 * 2 * NS], f8, name=f"e{t // 2}_{sl}",
                            tag="exp", bufs=3)
                    e2 = expt[t // 2]
                    nc.scalar.activation(jview(e2, 2 * NS)[:, t % 2, :],
                                         ps[:], AF.Exp, scale=e_scale,
                                         bias=ebias[:])

                def emit_ctx(sl, t2, cps, expt):
                    for h in range(2):
                        lhs = jview(v65[t2], H * 65)[
                            :, :, (2 * p + h) * 65:(2 * p + h) * 65 + 65]
                        nc.tensor.matmul(
                            cps[h][0:65, :], lhs,
                            jview(expt[t2], 2 * NS)[
                                :, :, h * NS:(h + 1) * NS],
                            perf_mode=DR,
                            start=(t2 == 0), stop=(t2 == T2 - 1))

                def emit_evict(sl, cps):
                    for h in range(2):
                        ps = cps[h]
                        stage = pA.tile([65, NS], f32, name=f"stg{h}{sl}",
                                        tag="rc", bufs=2)
                        nc.vector.tensor_copy(stage[64:65, :], ps[64:65, :])
                        c0 = 4 * (h * SL + sl)
                        nc.sync.dma_start(den128[:, c0:c0 + 4],
                                          stage[64:65, :])
                        if h == 0:
                            nc.vector.tensor_copy(
                                ctxU[0:HD, sl * NS:(sl + 1) * NS],
                                ps[0:HD, :])
                        else:
                            tmp = pA.tile([HD, NS], f32, name=f"ctmp{sl}",
                                          tag="ctmp", bufs=2)
                            nc.vector.tensor_copy(tmp[:], ps[0:HD, :])
                            nc.sync.dma_start(
                                ctxU[HD:P, sl * NS:(sl + 1) * NS], tmp[:])

                expt0 = {}
                cps0 = [psA.tile([P, NS], f32, name=f"cps{h}_0", tag="ctx",
                                 bufs=2) for h in range(2)]
                expt1 = {}
                cps1 = [psA.tile([P, NS], f32, name=f"cps{h}_1", tag="ctx",
                                 bufs=2) for h in range(2)]
                if p == 0:
                    # A: scores(sl0) + the whole V projection interleaved
                    for t in range(ST):
                        emit_scores(0, t, expt0)
                        for hc in range(4 * t, 4 * t + 4):
                            emit_v_chunk(hc)
                    # B: scores(sl1) + lagged ctx(sl0) per token pair
                    for t in range(ST + LAG):
                        if t < ST:
                            emit_scores(1, t, expt1)
                        if t >= LAG and (t - LAG) % 2 == 1:
                            emit_ctx(0, (t - LAG) // 2, cps0, expt0)
                    emit_evict(0, cps0)
                    # C: ctx(sl1) + QK(1) chunks
                    for t in range(ST):
                        if t % 2 == 1:
                            emit_ctx(1, t // 2, cps1, expt1)
                        if next_state is not None:
                            emit_qk_chain_part(p + 1, t // 2, t % 2,
                                               next_state)
                    emit_evict(1, cps1)
                    pV_cm.__exit__(None, None, None)
                else:
                    # A: scores(sl0) + QK(p+1) chunks 0-3 + lagged ctx(sl0)
                    for t in range(ST + LAG):
                        if t < ST:
                            emit_scores(0, t, expt0)
                            if next_state is not None and t < 4:
                                emit_qk_chain_part(p + 1, t // 2, t % 2,
                                                   next_state)
                        if t >= LAG and (t - LAG) % 2 == 1:
                            emit_ctx(0, (t - LAG) // 2, cps0, expt0)
                    emit_evict(0, cps0)
                    if pending is not None:
                        emit_normalize(*pending)
                    # B: scores(sl1) + QK(p+1) chunks 4-7 + lagged ctx(sl1)
                    for t in range(ST + LAG):
                        if t < ST:
                            emit_scores(1, t, expt1)
                            if next_state is not None and t < 4:
                                emit_qk_chain_part(p + 1, (t + 4) // 2,
                                                   t % 2, next_state)
                        if t >= LAG and (t - LAG) % 2 == 1:
                            emit_ctx(1, (t - LAG) // 2, cps1, expt1)
                    emit_evict(1, cps1)
                pending = (p, ctxU, den128)
                qk_state = next_state
            emit_normalize(*pending)
            pExp_cm.__exit__(None, None, None)

        # -------- Wo + LN1 + FFN merged scope (PE never starves) --------
        # hn = LN1(RSC*(x + bo) + ctxT2@wo2) * g1 + (be1 + b2); the RSC scale
        # on both residual and product cancels in LayerNorm. FFN1 half-0
        # chains are interleaved under the LN1/transpose epilogues of si 4-7
        # (the fp8 Wo matmuls alone cannot keep the PE fed).
        h_nat = []
        ht2 = [[pH.tile([P, NS], bf16, name=f"ht{hf}_{d}", tag="ht", bufs=2 * DT)
                for d in range(DT)] for hf in range(SL)]
        with tc.tile_pool(name="pWo", bufs=1) as pWo:
            psW_cm = tc.tile_pool(name="psW", bufs=1, space="PSUM")
            psW = psW_cm.__enter__()
            psU_cm = tc.tile_pool(name="psU", bufs=1, space="PSUM")
            psU = psU_cm.__enter__()
            bo_b = bcast_row(pWo, "bo", bo_d, D, tag="bc0")
            g1_b = bcast_row(pWo, "g1", g1_d, D, tag="bc1")
            be1_b = bcast_row(pWo, "be1", be1_d, D, tag="bc2")

            # residual tiles: RSC*(x + bo)
            xns = []
            for si in range(ST):
                xn = pWo.tile([P, D], f32, name=f"xn{si}", tag="xn", bufs=3)
                nc.sync.dma_start(xn[:], x_d[si * P:(si + 1) * P, :])
                nc.vector.tensor_add(xn[:], xn[:], bo_b[:])
                xns.append(xn)

            ut = [pWo.tile([P, S], bf16, name=f"ut{f}", tag="ut", bufs=FT)
                  for f in range(FT)]

            def emit_ffn1(f, hf):
                ps = psU.tile([P, NS], f32, name=f"u{f}_{hf}", tag="u",
                              bufs=4)
                for d in range(DT):
                    nc.tensor.matmul(
                        ps[:], w1[f][:, d * P:(d + 1) * P],
                        ht2[hf][d][:],
                        start=(d == 0), stop=(d == DT - 1))
                nc.scalar.activation(ut[f][:, hf * NS:(hf + 1) * NS],
                                     ps[:], AF.Relu, bias=b1c[:, f:f + 1])

            def ln_stats(pool, si, v, pfx):
                st = pool.tile([P, 8], f32, name=f"{pfx}st{si}", tag="st",
                               bufs=4)
                s1 = st[:, 2:3]
                s2 = st[:, 3:4]; mu = st[:, 4:5]; var = st[:, 5:6]
                rstd = st[:, 6:7]; nm = st[:, 7:8]
                scr = pool.tile([P, D], f32, name=f"{pfx}scr{si}", tag="scr",
                                bufs=2)
                nc.scalar.activation(scr[:], v[:], AF.Copy, accum_out=s1)
                nc.scalar.activation(scr[:], v[:], AF.Square, accum_out=s2)
                nc.vector.tensor_scalar_mul(mu, s1, 1.0 / D)
                nc.vector.tensor_scalar_mul(var, s2, 1.0 / D)
                nc.vector.tensor_mul(nm, mu, mu)
                nc.vector.tensor_sub(var, var, nm)
                nc.vector.tensor_scalar_add(var, var, EPS)
                nc.scalar.sqrt(var, var)
                nc.vector.reciprocal(rstd, var)
                nc.vector.tensor_mul(nm, mu, rstd)
                nc.vector.tensor_scalar_mul(nm, nm, -1.0)
                nc.scalar.activation(scr[:], v[:], AF.Identity,
                                     bias=nm, scale=rstd)
                return scr

            def emit_wo_si(si):
                pss = [psW.tile([P, ND], f32, name=f"c{si}_{n}", tag="c",
                                bufs=2) for n in range(DL)]
                for q2 in range(D2):
                    for n in range(DL):
                        nc.tensor.matmul(
                            pss[n][:],
                            jview(ctxT2[q2], S)[:, :, si * P:(si + 1) * P],
                            jview(wo2[q2], D)[:, :, n * ND:(n + 1) * ND],
                            perf_mode=DR,
                            start=(q2 == 0), stop=(q2 == D2 - 1))
                v = pWo.tile([P, D], f32, name=f"v{si}", tag="v", bufs=3)
                for n in range(DL):
                    nc.vector.tensor_add(
                        v[:, n * ND:(n + 1) * ND], pss[n][:],
                        xns[si][:, n * ND:(n + 1) * ND])
                scr = ln_stats(pWo, si, v, "ln1")
                hn = pH.tile([P, D], bf16, name=f"hn{si}", tag="hn", bufs=ST)
                nc.vector.tensor_mul(scr[:], scr[:], g1_b[:])
                nc.vector.tensor_add(hn[:], scr[:], be1_b[:])
                h_nat.append(hn)
                # h^T transposes for this si; copybacks alternate DVE/ACT
                for dd in range(DT):
                    ps = psW.tile([P, P], bf16, name=f"tp{si}_{dd}", tag="tp",
                                  bufs=2)
                    nc.tensor.transpose(
                        ps[:], hn[:, dd * P:(dd + 1) * P], identity[:])
                    dst = ht2[si // 4][dd][:, (si % 4) * P:(si % 4 + 1) * P]
                    if dd % 2 == 0:
                        nc.vector.tensor_copy(dst, ps[:])
                    else:
                        nc.scalar.copy(dst, ps[:])

            for si in range(ST):
                emit_wo_si(si)
                if 3 <= si <= 6:
                    for f in range(4 * (si - 3), 4 * (si - 3) + 4):
                        emit_ffn1(f, 0)

            g2_b = bcast_row(pWo, "g2", g2_d, D, tag="bc0")
            be2_b = bcast_row(pWo, "be2", be2_d, D, tag="bc1")
            for f in range(FT):
                emit_ffn1(f, 1)
            psU_cm.__exit__(None, None, None)
            psW_cm.__exit__(None, None, None)

            # ---- FFN2 si-major with per-si LN2 epilogue ----
            with tc.tile_pool(name="psY", bufs=1, space="PSUM") as psY:
                for si in range(ST):
                    pss = [psY.tile([P, ND], f32, name=f"y{si}_{n}", tag="y",
                                    bufs=6) for n in range(DL)]
                    for f in range(FT):
                        for n in range(DL):
                            nc.tensor.matmul(
                                pss[n][:],
                                ut[f][:, si * P:(si + 1) * P],
                                w2[f][:, n * ND:(n + 1) * ND],
                                start=(f == 0), stop=(f == FT - 1))
                    v = pWo.tile([P, D], f32, name=f"v2_{si}", tag="v",
                                 bufs=3)
                    for n in range(DL):
                        nc.vector.tensor_add(
                            v[:, n * ND:(n + 1) * ND], pss[n][:],
                            h_nat[si][:, n * ND:(n + 1) * ND])
                    scr = ln_stats(pWo, si, v, "ln2")
                    o = pWo.tile([P, D], f32, name=f"o{si}", tag="o", bufs=3)
                    nc.vector.tensor_mul(scr[:], scr[:], g2_b[:])
                    nc.vector.tensor_add(o[:], scr[:], be2_b[:])
                    nc.sync.dma_start(out_d[si * P:(si + 1) * P, :], o[:])
        pCtx_cm.__exit__(None, None, None)

    nc.compile()
    return nc


def pack_core_inputs(x_b, shared):
    """Per-core input map: batch element x_b + shared (prepacked) weights."""
    m = dict(shared)
    x_b = np.asarray(x_b, dtype=np.float32)
    m["x"] = np.ascontiguousarray(RSC * x_b)
    # x^T in fp8, dv-pair interleaved: [D2, P, 2, S]
    xT = np.ascontiguousarray(x_b.T)
    m["xT"] = np.ascontiguousarray(
        xT.reshape(D2, 2, P, S).transpose(0, 2, 1, 3).astype(np_f8))
    return m


def pack_shared(Wq, bq, Wk, bk, Wv, bv, Wo, bo, ln1_g, ln1_b, W1, b1, W2, b2,
                ln2_g, ln2_b):
    """Host-side layout packing of the replicated weights. fp8 weights are
    prescaled by WSC=32 (their sigma is ~1/32); be1 absorbs b2 exactly with
    b1 -= W1^T b2 compensating FFN1."""
    f = np.float32
    Wq = np.asarray(Wq, dtype=f); Wk = np.asarray(Wk, dtype=f)
    Wv = np.asarray(Wv, dtype=f); Wo = np.asarray(Wo, dtype=f)
    W1 = np.asarray(W1, dtype=f); W2 = np.asarray(W2, dtype=f)
    b1 = np.asarray(b1, dtype=f); b2 = np.asarray(b2, dtype=f)
    ln1_b = np.asarray(ln1_b, dtype=f)
    # [pair, p, d2, j, m]: DoubleRow dv-pair layout, contiguous per-pair DMA
    pack_qk = lambda W: np.ascontiguousarray(
        (WSC * W).reshape(D, H * HD).reshape(D2, 2, P, NP_, P).transpose(
            3, 2, 0, 1, 4).astype(np_f8))
    pack_dv = lambda W: np.ascontiguousarray(
        (WSC * W).reshape(D2, 2, P, D).transpose(0, 2, 1, 3).astype(np_f8))
    sel = np.zeros((SL, 4, P), dtype=np_bf16)
    for sl in range(SL):
        for m in range(P):
            sel[sl, (m // HD) * SL + sl, m] = 1.0
    be1_f = ln1_b + b2
    b1_f = (b1.astype(np.float64) -
            W1.astype(np.float64).T @ b2.astype(np.float64)).astype(f)
    return {
        "vones": np.ones((P, H), dtype=np_f8),
        "sel": sel,
        "Wq": pack_qk(Wq), "Wk": pack_qk(Wk),
        "Wv": pack_dv(Wv.reshape(D, D)),
        "Wo": pack_dv(Wo),
        "W1": np.ascontiguousarray(
            W1.reshape(DT, P, FT, P).transpose(2, 1, 0, 3)).astype(np_bf16),
        "W2": np.ascontiguousarray(W2).astype(np_bf16),
        "bqc": np.ascontiguousarray(WSC * np.asarray(bq, f).reshape(NP_, P).T),
        "bkc": np.ascontiguousarray(WSC * np.asarray(bk, f).reshape(NP_, P).T),
        "b1c": np.ascontiguousarray(b1_f.reshape(FT, P).T),
        "bv": np.ascontiguousarray(WSC * np.asarray(bv, f).reshape(D)),
        "bo": np.ascontiguousarray(RSC * np.asarray(bo, f)),
        "g1": np.ascontiguousarray(ln1_g, dtype=f),
        "be1": np.ascontiguousarray(be1_f),
        "g2": np.ascontiguousarray(ln2_g, dtype=f),
        "be2": np.ascontiguousarray(ln2_b, dtype=f),
    }


_NC_CACHE = {}


def get_nc():
    if "nc" not in _NC_CACHE:
        _NC_CACHE["nc"] = build_encoder(num_devices=8)
    return _NC_CACHE["nc"]


def kernel(x, Wq, bq, Wk, bk, Wv, bv, Wo, bo, ln1_g, ln1_b, W1, b1, W2, b2,
           ln2_g, ln2_b):
    x = np.asarray(x)
    assert x.shape == (B, S, D)
    shared = pack_shared(Wq, bq, Wk, bk, Wv, bv, Wo, bo, ln1_g, ln1_b,
                         W1, b1, W2, b2, ln2_g, ln2_b)
    in_maps = [pack_core_inputs(x[b], shared) for b in range(B)]
    nc = get_nc()
    res = bass_utils.run_bass_kernel_spmd(
        nc, in_maps, core_ids=list(range(B)), trace=False)
    return np.stack([res.results[b]["out"] for b in range(B)], axis=0)


# revision 32
# speedup vs baseline: 1.2295x; 1.0666x over previous
"""Transformer encoder layer (nn_Encoder) on 8 TRN2 NeuronCores.

Strategy: data-parallel over batch — B=8, one batch element per core, weights
replicated, no collectives. Per core a single Bass/Tile kernel computes the
whole layer.

Precision plan: the whole attention path (QKV projections, scores,
softmax-weighted context, Wo) runs in fp8e4m3 — its error is attenuated ~25x
by the residual (|attn_out| ~ 0.04 |x|). QKV/ctx/Wo use DoubleRow matmuls
(2 contraction tiles per instruction, 0.5 cycles/row). The FFN (whose output
is NOT small vs the residual) runs in bf16 at 1 cycle/row. All accumulation in
fp32 PSUM. fp8 operands are host-prescaled by 32 (weights are sigma~1/32 and
would hit the fp8 denormal floor); the combined 1024x scale on the attention
output is folded into the residual (x side is scaled 1024x on the host) and
neutralized by LayerNorm's scale invariance.

Layout: attention runs in the "transposed domain" ([feature, tokens]) so every
weight matmul uses natural weight layouts; softmax over tokens-on-partitions is
handled by appending a ones-column to V (denominator lands in the ctx matmul's
extra output row, M=65). Per (pair, slice) the denominator rows are packed
[128, 8] via reshape-DMAs so the reciprocal runs wide, laid back out as a
[2, NS] operand and broadcast across partitions with a tiny K=2 matmul against
a selection matrix, then applied in one full-width multiply per slice
(deferred one pair to keep the in-order PE queue from head-of-line blocking).
Wo/FFN products land in the natural domain where both LayerNorms reduce along
the free dim; the normalize runs on ACT via per-partition scale/bias. g1 and
b2 are folded on the host (W1 *= g1, b1 += W1^T be1, be1 += b2), so the
transposes and FFN1 consume the normalized h-hat straight from ACT while the
residual's full h is rebuilt off the critical path. Wo/W1/W2
are prefetched into SBUF during the attention phase, each read from HBM
exactly once. FFN1 is emitted per si token-column so it rides inside the Wo
phase right behind each tile's transposes (the fp8 Wo matmuls alone cannot
keep the PE fed under the LN1 epilogues); ReLU alternates ACT/DVE. FFN2 runs
si-major with per-si epilogues so only the last tile's epilogue trails the
final matmul.

Self-contained: hardcodes B=8, S=1024, D=1024, H=16, FF=2048, 8 cores.
"""
import math
import numpy as np
import ml_dtypes
from contextlib import ExitStack

import concourse.bass as bass
import concourse.tile as tile
from concourse import bacc, mybir
from concourse import bass_utils
from concourse.masks import make_identity

B = 8
S = 1024
D = 1024
H = 16
FF = 2048
P = 128
HD = 64
EPS = 1e-5
f32 = mybir.dt.float32
f32r = mybir.dt.float32r
bf16 = mybir.dt.bfloat16
f8 = mybir.dt.float8e4
np_bf16 = ml_dtypes.bfloat16
np_f8 = ml_dtypes.float8_e4m3
DR = mybir.MatmulPerfMode.DoubleRow
AF = mybir.ActivationFunctionType
ALU = mybir.AluOpType

NP_ = H // 2          # head pairs
ST = S // P           # token tiles
DT = D // P
D2 = DT // 2          # contraction pair-tiles over D
T2 = ST // 2          # token pair-tiles
FT = FF // P
NS = 512              # token slice width (matmul free dim)
SL = S // NS
ND = 512              # feature slice width
DL = D // ND

WSC = 32.0            # fp8 weight prescale (weights are sigma ~ 1/32)
RSC = WSC * WSC       # resulting scale on the attention output


def build_encoder(num_devices=8):
    # q,k arrive prescaled by WSC each: scores carry WSC^2; exp folds it into
    # its input scale. The extra -2ln2 bias keeps exp outputs < 240/4 (fp8e4m3
    # max is 240); softmax cancels any constant factor on e.
    e_scale = 1.0 / (math.sqrt(HD) * RSC)
    e_bias = -2.0 * math.log(2.0)
    nc = bacc.Bacc("TRN2", target_bir_lowering=False, debug=False,
                   enable_asserts=True, num_devices=num_devices)

    dram = lambda n, sh, dt: nc.dram_tensor(n, sh, dt, kind="ExternalInput").ap()
    xT_d = dram("xT", [D2, P, 2, S], f8)
    vones_d = dram("vones", [P, H], f8)
    sel_d = dram("sel", [SL, 4, P], bf16)
    x_d = dram("x", [S, D], f32)
    wq_d = dram("Wq", [NP_, P, D2, 2, P], f8)
    wk_d = dram("Wk", [NP_, P, D2, 2, P], f8)
    wv_d = dram("Wv", [D2, P, 2, D], f8)
    wo_d = dram("Wo", [D2, P, 2, D], f8)
    w1_d = dram("W1", [FT, P, DT, P], bf16)
    w2_d = dram("W2", [FF, D], bf16)
    bqc_d = dram("bqc", [P, NP_], f32)
    bkc_d = dram("bkc", [P, NP_], f32)
    b1c_d = dram("b1c", [P, FT], f32)
    bv_d = dram("bv", [D], f32)
    bo_d = dram("bo", [D], f32)
    g1_d = dram("g1", [D], f32)
    be1_d = dram("be1", [D], f32)
    g2_d = dram("g2", [D], f32)
    be2_d = dram("be2", [D], f32)
    out_d = nc.dram_tensor("out", [S, D], f32, kind="ExternalOutput").ap()

    with tile.TileContext(nc) as tc, ExitStack() as octx:
        const = octx.enter_context(tc.tile_pool(name="const", bufs=1))
        identity = const.tile([P, P], bf16, name="identity")
        make_identity(nc, identity)
        ebias = const.tile([P, 1], f32, name="ebias")
        nc.gpsimd.memset(ebias[:], e_bias)

        # long-lived weight pool (prefetched during attention) + h tiles
        pW = octx.enter_context(tc.tile_pool(name="pW", bufs=1))
        pH = octx.enter_context(tc.tile_pool(name="pH", bufs=1))

        def bcast_row(pool, name, src_row, width, tag=None):
            r = pool.tile([1, width], f32, name=f"{name}_r", tag="bcr", bufs=1)
            nc.sync.dma_start(r[:], src_row[None, :])
            b = pool.tile([P, width], f32, name=f"{name}_b",
                          tag=(tag or f"{name}_b"))
            nc.gpsimd.partition_broadcast(b[:], r[:])
            return b

        jview = lambda ap, w: ap.rearrange("p (j c) -> p j c", j=2)

        # ctxT pool (attention -> Wo), fp8 dv-pair tiles [P, 2, S]
        pCtx_cm = tc.tile_pool(name="pCtx", bufs=1)
        pCtx = pCtx_cm.__enter__()

        # ---------------- attention scope ----------------
        with tc.tile_pool(name="pA", bufs=1) as pA, \
             tc.tile_pool(name="psA", bufs=1, space="PSUM") as psA:

            # pair-0 Q/K weights first so QK(0) matmuls start ASAP
            wq0 = pA.tile([P, D2 * 2 * P], f8, name="wq0", tag="wq", bufs=2)
            nc.sync.dma_start(
                wq0[:].rearrange("p (d j m) -> p d j m", j=2, m=P), wq_d[0])
            wk0 = pA.tile([P, D2 * 2 * P], f8, name="wk0", tag="wk", bufs=2)
            nc.sync.dma_start(
                wk0[:].rearrange("p (d j m) -> p d j m", j=2, m=P), wk_d[0])

            # x^T fp8 pair tiles [P, 2, S]
            xt2 = []
            for d2 in range(D2):
                t = pA.tile([P, 2 * S], f8, name=f"xt{d2}", tag="xt", bufs=D2)
                nc.sync.dma_start(jview(t, S), xT_d[d2])
                xt2.append(t)

            # ---- V projection weights ----
            pExp_cm = tc.tile_pool(name="pExp", bufs=1)
            pExp = pExp_cm.__enter__()
            pV_cm = tc.tile_pool(name="pV", bufs=1)
            pV = pV_cm.__enter__()
            wv2 = []
            for d2 in range(D2):
                t = pV.tile([P, 2 * D], f8, name=f"wv{d2}", tag="wv", bufs=D2)
                nc.sync.dma_start(jview(t, D), wv_d[d2])
                wv2.append(t)

            # V65 token-pair tiles: [128, 2, H*65], ones at [:, :, 65h+64]
            v65 = []
            for t2 in range(T2):
                v = pA.tile([P, 2 * H * 65], f8, name=f"v65_{t2}", tag="v65",
                            bufs=T2)
                nc.gpsimd.memset(
                    jview(v, H * 65).rearrange(
                        "p j (h c) -> p j h c", c=65)[:, :, :, 64:65], 1.0)
                v65.append(v)

            # small consts (needed only after the first QK chains land)
            bqc = const.tile([P, NP_], f32, name="bqc")
            nc.sync.dma_start(bqc[:], bqc_d)
            bkc = const.tile([P, NP_], f32, name="bkc")
            nc.sync.dma_start(bkc[:], bkc_d)
            b1c = const.tile([P, FT], f32, name="b1c")
            nc.sync.dma_start(b1c[:], b1c_d)
            selt = const.tile([4, SL * P], bf16, name="selt")
            for sl in range(SL):
                nc.sync.dma_start(selt[:, sl * P:(sl + 1) * P], sel_d[sl])
            bv_b = bcast_row(pA, "bv", bv_d, D)

            # Wo prefetch (fp8 dv-pair tiles [P, 2, D])
            wo2 = []
            for q2 in range(D2):
                t = pW.tile([P, 2 * D], f8, name=f"wo{q2}", tag="wo", bufs=D2)
                nc.sync.dma_start(jview(t, D), wo_d[q2])
                wo2.append(t)

            # W1/W2 resident tiles; DMAs emitted later in the pair loop
            w1 = [pW.tile([P, DT * P], bf16, name=f"w1_{f}", tag="w1", bufs=FT)
                  for f in range(FT)]
            w2 = [pW.tile([P, D], bf16, name=f"w2_{f}", tag="w2", bufs=FT)
                  for f in range(FT)]

            def emit_w1_loads():
                for f in range(FT):
                    nc.sync.dma_start(
                        w1[f][:].rearrange("p (dt q) -> p dt q", q=P),
                        w1_d[f])

            def emit_w2_loads():
                for f in range(FT):
                    nc.sync.dma_start(w2[f][:], w2_d[f * P:(f + 1) * P, :])

            hpn = ND // HD
            v_state = {}

            def emit_v_chunk(hc):
                """Half-chunk hc of the V projection (chain = hc//2)."""
                chain = hc // 2
                part = hc % 2
                t, n = chain // DL, chain % DL
                if part == 0:
                    v_state[chain] = psA.tile(
                        [P, ND], f32, name=f"vps{t}_{n}", tag="vqk", bufs=2)
                ps = v_state[chain]
                for d2 in range(2 * part, 2 * part + 2):
                    nc.tensor.matmul(
                        ps[:], jview(xt2[d2], S)[:, :, t * P:(t + 1) * P],
                        jview(wv2[d2], D)[:, :, n * ND:(n + 1) * ND],
                        perf_mode=DR, start=(d2 == 0), stop=(d2 == D2 - 1))
                if part == 1:
                    dst = jview(v65[t // 2], H * 65).rearrange(
                        "p j (h c) -> p j h c", c=65)[
                        :, t % 2, n * hpn:(n + 1) * hpn, 0:64]
                    srcv = ps[:].rearrange("p (h k) -> p h k", k=HD)
                    bvs = bv_b[:, n * ND:(n + 1) * ND].rearrange(
                        "p (h k) -> p h k", k=HD)
                    nc.vector.tensor_add(dst, srcv, bvs)

            # ---- attention per head pair ----
            ctxT2 = [pCtx.tile([P, 2 * S], f8, name=f"ctxT{q2}", tag="ctxT",
                               bufs=D2) for q2 in range(D2)]

            def emit_normalize(p, ctxU, den128):
                """Deferred softmax-normalize of pair p's ctx. The
                denominators live as [128, 16] so the reciprocal runs wide;
                a reshape DMA lays them back out as the [4, NS] rcb moving
                operand."""
                denr = pA.tile([P, 16], bf16, name=f"denr_{p}", tag="denr",
                               bufs=2)
                with nc.allow_low_precision("softmax denom recip in bf16"):
                    nc.vector.reciprocal(denr[:], den128[:])
                den4r = pA.tile([4, NS], bf16, name=f"den4r_{p}", tag="den4r",
                                bufs=2)
                for r in range(4):
                    nc.sync.dma_start(den4r[r:r + 1, :], denr[:, 4 * r:4 * r + 4])
                for sl in range(SL):
                    rcb = psA.tile([P, NS], f32, name=f"rcb{p}_{sl}",
                                   tag="vqk", bufs=2)
                    nc.tensor.matmul(rcb[:], selt[:, sl * P:(sl + 1) * P],
                                     den4r[:], start=True, stop=True)
                    nc.vector.tensor_mul(
                        jview(ctxT2[p // 2], S)[
                            :, p % 2, sl * NS:(sl + 1) * NS],
                        ctxU[:, sl * NS:(sl + 1) * NS], rcb[:])

            def emit_qk_chain_part(p, chain, part, state):
                """Emit 2 of the 4 DoubleRow accumulation matmuls of QK chain
                (chain: 0..3 = Q-sl0, Q-sl1, K-sl0, K-sl1) for pair p."""
                wt, bc, dst = state["ops"][chain // 2]
                sl = chain % 2
                if part == 0:
                    state[chain] = psA.tile(
                        [P, NS], f32, name=f"qk{p}_{chain}", tag="vqk", bufs=2)
                ps = state[chain]
                wtv = wt[:].rearrange("p (d j m) -> p d j m", j=2, m=P)
                for d2 in range(2 * part, 2 * part + 2):
                    nc.tensor.matmul(
                        ps[:], wtv[:, d2],
                        jview(xt2[d2], S)[:, :, sl * NS:(sl + 1) * NS],
                        perf_mode=DR, start=(d2 == 0), stop=(d2 == D2 - 1))
                if part == 1:
                    nc.vector.tensor_scalar(
                        out=dst[:, sl * NS:(sl + 1) * NS], in0=ps[:],
                        scalar1=bc[:, p:p + 1], scalar2=None, op0=ALU.add)

            def make_qk_state(p):
                if p == 0:
                    wqt, wkt = wq0, wk0
                else:
                    wqt = pA.tile([P, D2 * 2 * P], f8, name=f"wq{p}",
                                  tag="wq", bufs=2)
                    nc.sync.dma_start(
                        wqt[:].rearrange("p (d j m) -> p d j m", j=2, m=P),
                        wq_d[p])
                    wkt = pA.tile([P, D2 * 2 * P], f8, name=f"wk{p}",
                                  tag="wk", bufs=2)
                    nc.sync.dma_start(
                        wkt[:].rearrange("p (d j m) -> p d j m", j=2, m=P),
                        wk_d[p])
                qt = pA.tile([P, S], f8, name=f"qt{p}", tag="qt", bufs=2)
                kt = pA.tile([P, S], f8, name=f"kt{p}", tag="kt", bufs=2)
                return {"ops": ((wqt, bqc, qt), (wkt, bkc, kt)),
                        "qt": qt, "kt": kt}

            LAG = 2
            qk_state = make_qk_state(0)
            for chain in range(4):
                for part in range(2):
                    emit_qk_chain_part(0, chain, part, qk_state)

            pending = None
            for p in range(NP_):
                if p == 2:
                    emit_w1_loads()
                if p == 5:
                    emit_w2_loads()
                qt, kt = qk_state["qt"], qk_state["kt"]
                next_state = make_qk_state(p + 1) if p + 1 < NP_ else None

                ctxU = pA.tile([P, S], f32, name=f"ctxU{p}", tag="ctxU",
                               bufs=2)
                den128 = pA.tile([P, 16], f32, name=f"den128_{p}", tag="den",
                                 bufs=2)

                def emit_scores(sl, t, expt):
                    ps = psA.tile([P, 2 * NS], f32, name=f"sc{t}_{sl}",
                                  tag="sc", bufs=2)
                    for h in range(2):
                        nc.tensor.matmul(
                            ps[:, h * NS:(h + 1) * NS],
                            kt[h * HD:(h + 1) * HD, t * P:(t + 1) * P],
                            qt[h * HD:(h + 1) * HD, sl * NS:(sl + 1) * NS],
                            start=True, stop=True,
                            tile_position=(h * HD, 0))
                    if t % 2 == 0:
                        expt[t // 2] = pExp.tile(
                            [P, 2 * 2 * NS], f8, name=f"e{t // 2}_{sl}",
                            tag="exp", bufs=3)
                    e2 = expt[t // 2]
                    nc.scalar.activation(jview(e2, 2 * NS)[:, t % 2, :],
                                         ps[:], AF.Exp, scale=e_scale,
                                         bias=ebias[:])

                def emit_ctx(sl, t2, cps, expt):
                    for h in range(2):
                        lhs = jview(v65[t2], H * 65)[
                            :, :, (2 * p + h) * 65:(2 * p + h) * 65 + 65]
                        nc.tensor.matmul(
                            cps[h][0:65, :], lhs,
                            jview(expt[t2], 2 * NS)[
                                :, :, h * NS:(h + 1) * NS],
                            perf_mode=DR,
                            start=(t2 == 0), stop=(t2 == T2 - 1))

                def emit_evict(sl, cps):
                    for h in range(2):
                        ps = cps[h]
                        stage = pA.tile([65, NS], f32, name=f"stg{h}{sl}",
                                        tag="rc", bufs=2)
                        nc.vector.tensor_copy(stage[64:65, :], ps[64:65, :])
                        c0 = 4 * (h * SL + sl)
                        nc.sync.dma_start(den128[:, c0:c0 + 4],
                                          stage[64:65, :])
                        if h == 0:
                            nc.vector.tensor_copy(
                                ctxU[0:HD, sl * NS:(sl + 1) * NS],
                                ps[0:HD, :])
                        else:
                            tmp = pA.tile([HD, NS], f32, name=f"ctmp{sl}",
                                          tag="ctmp", bufs=2)
                            nc.vector.tensor_copy(tmp[:], ps[0:HD, :])
                            nc.sync.dma_start(
                                ctxU[HD:P, sl * NS:(sl + 1) * NS], tmp[:])

                expt0 = {}
                cps0 = [psA.tile([P, NS], f32, name=f"cps{h}_0", tag="ctx",
                                 bufs=2) for h in range(2)]
                expt1 = {}
                cps1 = [psA.tile([P, NS], f32, name=f"cps{h}_1", tag="ctx",
                                 bufs=2) for h in range(2)]
                if p == 0:
                    # A: scores(sl0) + the whole V projection interleaved
                    for t in range(ST):
                        emit_scores(0, t, expt0)
                        for hc in range(4 * t, 4 * t + 4):
                            emit_v_chunk(hc)
                    # B: scores(sl1) + lagged ctx(sl0) per token pair
                    for t in range(ST + LAG):
                        if t < ST:
                            emit_scores(1, t, expt1)
                        if t >= LAG and (t - LAG) % 2 == 1:
                            emit_ctx(0, (t - LAG) // 2, cps0, expt0)
                    emit_evict(0, cps0)
                    # C: ctx(sl1) + QK(1) chunks
                    for t in range(ST):
                        if t % 2 == 1:
                            emit_ctx(1, t // 2, cps1, expt1)
                        if next_state is not None:
                            emit_qk_chain_part(p + 1, t // 2, t % 2,
                                               next_state)
                    emit_evict(1, cps1)
                    pV_cm.__exit__(None, None, None)
                else:
                    # A: scores(sl0) + QK(p+1) chunks 0-3 + lagged ctx(sl0)
                    for t in range(ST + LAG):
                        if t < ST:
                            emit_scores(0, t, expt0)
                            if next_state is not None and t < 4:
                                emit_qk_chain_part(p + 1, t // 2, t % 2,
                                                   next_state)
                        if t >= LAG and (t - LAG) % 2 == 1:
                            emit_ctx(0, (t - LAG) // 2, cps0, expt0)
                    emit_evict(0, cps0)
                    if pending is not None:
                        emit_normalize(*pending)
                    # B: scores(sl1) + QK(p+1) chunks 4-7 + lagged ctx(sl1)
                    for t in range(ST + LAG):
                        if t < ST:
                            emit_scores(1, t, expt1)
                            if next_state is not None and t < 4:
                                emit_qk_chain_part(p + 1, (t + 4) // 2,
                                                   t % 2, next_state)
                        if t >= LAG and (t - LAG) % 2 == 1:
                            emit_ctx(1, (t - LAG) // 2, cps1, expt1)
                    emit_evict(1, cps1)
                pending = (p, ctxU, den128)
                qk_state = next_state
            emit_normalize(*pending)
            pExp_cm.__exit__(None, None, None)

        # -------- Wo + LN1 + FFN merged scope (PE never starves) --------
        # hn = LN1(RSC*(x + bo) + ctxT2@wo2) * g1 + (be1 + b2); the RSC scale
        # on both residual and product cancels in LayerNorm. FFN1 half-0
        # chains are interleaved under the LN1/transpose epilogues of si 4-7
        # (the fp8 Wo matmuls alone cannot keep the PE fed).
        h_nat = []
        ht2 = [[pH.tile([P, NS], bf16, name=f"ht{hf}_{d}", tag="ht", bufs=2 * DT)
                for d in range(DT)] for hf in range(SL)]
        with tc.tile_pool(name="pWo", bufs=1) as pWo:
            psW_cm = tc.tile_pool(name="psW", bufs=1, space="PSUM")
            psW = psW_cm.__enter__()
            psU_cm = tc.tile_pool(name="psU", bufs=1, space="PSUM")
            psU = psU_cm.__enter__()
            bo_b = bcast_row(pWo, "bo", bo_d, D, tag="bc0")
            g1_b = bcast_row(pWo, "g1", g1_d, D, tag="bc1")
            be1_b = bcast_row(pWo, "be1", be1_d, D, tag="bc2")

            # residual tiles: RSC*(x + bo)
            xns = []
            for si in range(ST):
                xn = pWo.tile([P, D], f32, name=f"xn{si}", tag="xn", bufs=3)
                nc.sync.dma_start(xn[:], x_d[si * P:(si + 1) * P, :])
                nc.vector.tensor_add(xn[:], xn[:], bo_b[:])
                xns.append(xn)

            ut = [pWo.tile([P, S], bf16, name=f"ut{f}", tag="ut", bufs=FT)
                  for f in range(FT)]

            def emit_ffn1(f, si):
                """FFN1 chain for one si token-column (moving free = 128),
                so FFN1 rides inside the Wo phase right behind si's
                transposes. ReLU+bias alternates ACT/DVE to balance load."""
                ps = psU.tile([P, P], f32, name=f"u{f}_{si}", tag="u",
                              bufs=4)
                hf, c0 = si // 4, (si % 4) * P
                for d in range(DT):
                    nc.tensor.matmul(
                        ps[:], w1[f][:, d * P:(d + 1) * P],
                        ht2[hf][d][:, c0:c0 + P],
                        start=(d == 0), stop=(d == DT - 1))
                dst = ut[f][:, si * P:(si + 1) * P]
                if f % 2 == 0:
                    nc.scalar.activation(dst, ps[:], AF.Relu,
                                         bias=b1c[:, f:f + 1])
                else:
                    nc.vector.tensor_scalar(
                        out=dst, in0=ps[:], scalar1=b1c[:, f:f + 1],
                        scalar2=0.0, op0=ALU.add, op1=ALU.max)

            def ln_stats(pool, si, v, pfx):
                st = pool.tile([P, 8], f32, name=f"{pfx}st{si}", tag="st",
                               bufs=4)
                s1 = st[:, 2:3]
                s2 = st[:, 3:4]; mu = st[:, 4:5]; var = st[:, 5:6]
                rstd = st[:, 6:7]; nm = st[:, 7:8]
                scr = pool.tile([P, D], f32, name=f"{pfx}scr{si}", tag="scr",
                                bufs=2)
                nc.scalar.activation(scr[:], v[:], AF.Copy, accum_out=s1)
                nc.scalar.activation(scr[:], v[:], AF.Square, accum_out=s2)
                nc.vector.tensor_scalar_mul(mu, s1, 1.0 / D)
                nc.vector.tensor_scalar_mul(var, s2, 1.0 / D)
                nc.vector.tensor_mul(nm, mu, mu)
                nc.vector.tensor_sub(var, var, nm)
                nc.vector.tensor_scalar_add(var, var, EPS)
                nc.scalar.sqrt(var, var)
                nc.vector.reciprocal(rstd, var)
                nc.vector.tensor_mul(nm, mu, rstd)
                nc.vector.tensor_scalar_mul(nm, nm, -1.0)
                nc.scalar.activation(scr[:], v[:], AF.Identity,
                                     bias=nm, scale=rstd)
                return scr

            def emit_wo_si(si):
                pss = [psW.tile([P, ND], f32, name=f"c{si}_{n}", tag="c",
                                bufs=2) for n in range(DL)]
                for q2 in range(D2):
                    for n in range(DL):
                        nc.tensor.matmul(
                            pss[n][:],
                            jview(ctxT2[q2], S)[:, :, si * P:(si + 1) * P],
                            jview(wo2[q2], D)[:, :, n * ND:(n + 1) * ND],
                            perf_mode=DR,
                            start=(q2 == 0), stop=(q2 == D2 - 1))
                v = pWo.tile([P, D], f32, name=f"v{si}", tag="v", bufs=3)
                for n in range(DL):
                    nc.vector.tensor_add(
                        v[:, n * ND:(n + 1) * ND], pss[n][:],
                        xns[si][:, n * ND:(n + 1) * ND])
                scr = ln_stats(pWo, si, v, "ln1")
                hn = pH.tile([P, D], bf16, name=f"hn{si}", tag="hn", bufs=ST)
                nc.vector.tensor_mul(scr[:], scr[:], g1_b[:])
                nc.vector.tensor_add(hn[:], scr[:], be1_b[:])
                h_nat.append(hn)
                # h^T transposes for this si; copybacks alternate DVE/ACT
                for dd in range(DT):
                    ps = psW.tile([P, P], bf16, name=f"tp{si}_{dd}", tag="tp",
                                  bufs=2)
                    nc.tensor.transpose(
                        ps[:], hn[:, dd * P:(dd + 1) * P], identity[:])
                    dst = ht2[si // 4][dd][:, (si % 4) * P:(si % 4 + 1) * P]
                    if dd % 2 == 0:
                        nc.vector.tensor_copy(dst, ps[:])
                    else:
                        nc.scalar.copy(dst, ps[:])

            for si in range(ST):
                emit_wo_si(si)
                for f in range(FT):
                    emit_ffn1(f, si)

            g2_b = bcast_row(pWo, "g2", g2_d, D, tag="bc0")
            be2_b = bcast_row(pWo, "be2", be2_d, D, tag="bc1")
            psU_cm.__exit__(None, None, None)
            psW_cm.__exit__(None, None, None)

            # ---- FFN2 si-major with per-si LN2 epilogue ----
            with tc.tile_pool(name="psY", bufs=1, space="PSUM") as psY:
                for si in range(ST):
                    pss = [psY.tile([P, ND], f32, name=f"y{si}_{n}", tag="y",
                                    bufs=6) for n in range(DL)]
                    for f in range(FT):
                        for n in range(DL):
                            nc.tensor.matmul(
                                pss[n][:],
                                ut[f][:, si * P:(si + 1) * P],
                                w2[f][:, n * ND:(n + 1) * ND],
                                start=(f == 0), stop=(f == FT - 1))
                    v = pWo.tile([P, D], f32, name=f"v2_{si}", tag="v",
                                 bufs=3)
                    for n in range(DL):
                        nc.vector.tensor_add(
                            v[:, n * ND:(n + 1) * ND], pss[n][:],
                            h_nat[si][:, n * ND:(n + 1) * ND])
                    scr = ln_stats(pWo, si, v, "ln2")
                    o = pWo.tile([P, D], f32, name=f"o{si}", tag="o", bufs=3)
                    nc.vector.tensor_mul(scr[:], scr[:], g2_b[:])
                    nc.vector.tensor_add(o[:], scr[:], be2_b[:])
                    nc.sync.dma_start(out_d[si * P:(si + 1) * P, :], o[:])
        pCtx_cm.__exit__(None, None, None)

    nc.compile()
    return nc


def pack_core_inputs(x_b, shared):
    """Per-core input map: batch element x_b + shared (prepacked) weights."""
    m = dict(shared)
    x_b = np.asarray(x_b, dtype=np.float32)
    m["x"] = np.ascontiguousarray(RSC * x_b)
    # x^T in fp8, dv-pair interleaved: [D2, P, 2, S]
    xT = np.ascontiguousarray(x_b.T)
    m["xT"] = np.ascontiguousarray(
        xT.reshape(D2, 2, P, S).transpose(0, 2, 1, 3).astype(np_f8))
    return m


def pack_shared(Wq, bq, Wk, bk, Wv, bv, Wo, bo, ln1_g, ln1_b, W1, b1, W2, b2,
                ln2_g, ln2_b):
    """Host-side layout packing of the replicated weights. fp8 weights are
    prescaled by WSC=32 (their sigma is ~1/32); be1 absorbs b2 exactly with
    b1 -= W1^T b2 compensating FFN1."""
    f = np.float32
    Wq = np.asarray(Wq, dtype=f); Wk = np.asarray(Wk, dtype=f)
    Wv = np.asarray(Wv, dtype=f); Wo = np.asarray(Wo, dtype=f)
    W1 = np.asarray(W1, dtype=f); W2 = np.asarray(W2, dtype=f)
    b1 = np.asarray(b1, dtype=f); b2 = np.asarray(b2, dtype=f)
    ln1_b = np.asarray(ln1_b, dtype=f)
    # [pair, p, d2, j, m]: DoubleRow dv-pair layout, contiguous per-pair DMA
    pack_qk = lambda W: np.ascontiguousarray(
        (WSC * W).reshape(D, H * HD).reshape(D2, 2, P, NP_, P).transpose(
            3, 2, 0, 1, 4).astype(np_f8))
    pack_dv = lambda W: np.ascontiguousarray(
        (WSC * W).reshape(D2, 2, P, D).transpose(0, 2, 1, 3).astype(np_f8))
    sel = np.zeros((SL, 4, P), dtype=np_bf16)
    for sl in range(SL):
        for m in range(P):
            sel[sl, (m // HD) * SL + sl, m] = 1.0
    be1_f = ln1_b + b2
    b1_f = (b1.astype(np.float64) -
            W1.astype(np.float64).T @ b2.astype(np.float64)).astype(f)
    return {
        "vones": np.ones((P, H), dtype=np_f8),
        "sel": sel,
        "Wq": pack_qk(Wq), "Wk": pack_qk(Wk),
        "Wv": pack_dv(Wv.reshape(D, D)),
        "Wo": pack_dv(Wo),
        "W1": np.ascontiguousarray(
            W1.reshape(DT, P, FT, P).transpose(2, 1, 0, 3)).astype(np_bf16),
        "W2": np.ascontiguousarray(W2).astype(np_bf16),
        "bqc": np.ascontiguousarray(WSC * np.asarray(bq, f).reshape(NP_, P).T),
        "bkc": np.ascontiguousarray(WSC * np.asarray(bk, f).reshape(NP_, P).T),
        "b1c": np.ascontiguousarray(b1_f.reshape(FT, P).T),
        "bv": np.ascontiguousarray(WSC * np.asarray(bv, f).reshape(D)),
        "bo": np.ascontiguousarray(RSC * np.asarray(bo, f)),
        "g1": np.ascontiguousarray(ln1_g, dtype=f),
        "be1": np.ascontiguousarray(be1_f),
        "g2": np.ascontiguousarray(ln2_g, dtype=f),
        "be2": np.ascontiguousarray(ln2_b, dtype=f),
    }


_NC_CACHE = {}


def get_nc():
    if "nc" not in _NC_CACHE:
        _NC_CACHE["nc"] = build_encoder(num_devices=8)
    return _NC_CACHE["nc"]


def kernel(x, Wq, bq, Wk, bk, Wv, bv, Wo, bo, ln1_g, ln1_b, W1, b1, W2, b2,
           ln2_g, ln2_b):
    x = np.asarray(x)
    assert x.shape == (B, S, D)
    shared = pack_shared(Wq, bq, Wk, bk, Wv, bv, Wo, bo, ln1_g, ln1_b,
                         W1, b1, W2, b2, ln2_g, ln2_b)
    in_maps = [pack_core_inputs(x[b], shared) for b in range(B)]
    nc = get_nc()
    res = bass_utils.run_bass_kernel_spmd(
        nc, in_maps, core_ids=list(range(B)), trace=False)
    return np.stack([res.results[b]["out"] for b in range(B)], axis=0)
